# revision 13
# baseline (speedup 1.0000x reference)
"""Trainium2 Bass kernel for nn_CifarBaselineSNN.

conv1(3->64,3x3,p1) -> BN -> LIF -> avgpool2
conv2(64->128,3x3,p1) -> BN -> LIF -> avgpool2
fc1(8192->256) -> LIF -> fc2(256->10)+b
T=8, B=128. Data-parallel over B across 8 NeuronCores (16 samples/core);
BN statistics are global over the batch -> 2 small AllReduces.

Numerics: convolutions run as bf16 hi/lo weight-split matmuls accumulating in
fp32 PSUM (inputs to conv2 are pooled spikes, exactly representable in bf16;
conv1 inputs are hi/lo split too). LIF state uses the 2^t scaling trick so one
fused DVE op does decay+charge+reset per step.
"""

import sys
import os

for _p in ("/opt/trn_rl_repo", "/root/.axon_site/_ro/trn_rl_repo"):
    if os.path.isdir(_p) and _p not in sys.path:
        sys.path.append(_p)

import numpy as np

import concourse.bass as bass
import concourse.mybir as mybir
import concourse.tile as tile
from concourse import bacc
from concourse import bass_utils
from concourse import dve_ops as _dops
from concourse.dve_uop import DveOpSpec
from concourse.dve_spec import (
    Spec, Src0, Src1, C0, C1, C2, Zero, select, lower, _has_src1,
)

F32 = mybir.dt.float32
BF16 = mybir.dt.bfloat16
AF = mybir.ActivationFunctionType
ALU = mybir.AluOpType

T = 8
B_FULL = 128
N_CORES = 8
BL = B_FULL // N_CORES  # 16 samples per core
EPS = 1e-5


# --------------------------------------------------------------------------
# Custom DVE ops (fused LIF pieces)
# --------------------------------------------------------------------------

def _register_op(name, spec, ref):
    for op in _dops.OPS:
        if op.name == name:
            return op
    spec = Spec(body=spec.body, accum=spec.accum, accum_init=spec.accum_init,
                reference=ref)
    shas = {}
    for ver in ("v3", "v4"):
        s = DveOpSpec(name=name, opcode=0, uops=lower(spec, ver=ver),
                      rd1_en=_has_src1(spec))
        shas[ver] = s.sha(ver)
    op = _dops.DveOp(name, spec, subdim=False, uops_sha=shas)
    _dops.OPS.append(op)
    _dops.CUSTOM_DVE_SPECS[name] = spec
    _dops._SUB_OPCODE_FOR_NAME[name] = max(_dops._SUB_OPCODE_FOR_NAME.values()) + 1
    return op


# A_t = (A_{t-1} if A_{t-1} < theta_{t-1} else 0) + y*scale + bias
LIF_CHARGE = _register_op(
    "LIF_CHARGE_SNN",
    Spec(body=select(Src0 >= C2, Zero, Src0) + Src1 * C0 + C1),
    lambda in0, in1, s0, s1, imm2: np.where(in0 >= imm2, 0.0, in0) + in1 * s0 + s1,
)

# spike counts over horizontal pairs: (a>=th) + (b>=th)   (values 0/1/2)
SPIKE_HPOOL = _register_op(
    "SPIKE_HPOOL_SNN",
    Spec(body=(Src0 >= C0) + (Src1 >= C0)),
    lambda in0, in1, s0, s1, imm2: (in0 >= s0).astype(np.float32)
    + (in1 >= s0).astype(np.float32),
)

# pooled = (hpA + hpB) * 0.25
VPOOL_SCALE = _register_op(
    "VPOOL_SCALE_SNN",
    Spec(body=(Src0 + Src1) * C1),
    lambda in0, in1, s0, s1, imm2: (in0 + in1) * s1,
)

# plain spike: (a >= th)
SPIKE_GE = _register_op(
    "SPIKE_GE_SNN",
    Spec(body=(Src0 >= C0) + Zero),
    lambda in0, s0, s1, imm2: (in0 >= s0).astype(np.float32),
)


# --------------------------------------------------------------------------
# Kernel build
# --------------------------------------------------------------------------

def _stats_to_scale_bias(nc, pool, tot, g_dr, b_dr, n_count, nch, out_scale, out_bias):
    """tot: [nch,2] SBUF (sum, sumsq). Writes out_scale/out_bias [nch,8]:
    scale[:,t] = gamma*rstd*2^t ; bias[:,t] = (beta - mu*gamma*rstd)*2^t."""
    mu = pool.tile([nch, 1], F32)
    nc.vector.tensor_scalar_mul(mu[:], tot[:, 0:1], 1.0 / n_count)
    e2 = pool.tile([nch, 1], F32)
    nc.vector.tensor_scalar_mul(e2[:], tot[:, 1:2], 1.0 / n_count)
    var = pool.tile([nch, 1], F32)
    nc.vector.tensor_tensor(var[:], mu[:], mu[:], ALU.mult)
    nc.vector.tensor_tensor(var[:], e2[:], var[:], ALU.subtract)
    nc.vector.tensor_scalar_add(var[:], var[:], float(EPS))
    std = pool.tile([nch, 1], F32)
    nc.scalar.activation(std[:], var[:], AF.Sqrt, bias=0.0, scale=1.0)
    rstd = pool.tile([nch, 1], F32)
    nc.vector.reciprocal(rstd[:], std[:])
    gam = pool.tile([nch, 1], F32)
    nc.sync.dma_start(gam[:], g_dr.ap()[:, None])
    bet = pool.tile([nch, 1], F32)
    nc.sync.dma_start(bet[:], b_dr.ap()[:, None])
    gr = pool.tile([nch, 1], F32)
    nc.vector.tensor_tensor(gr[:], gam[:], rstd[:], ALU.mult)
    bb = pool.tile([nch, 1], F32)  # beta - mu*gr
    nc.vector.tensor_tensor(bb[:], mu[:], gr[:], ALU.mult)
    nc.vector.tensor_tensor(bb[:], bet[:], bb[:], ALU.subtract)
    for t in range(T):
        nc.vector.tensor_scalar_mul(out_scale[:nch, t : t + 1], gr[:], float(2.0**t))
        nc.vector.tensor_scalar_mul(out_bias[:nch, t : t + 1], bb[:], float(2.0**t))


def _allreduce(nc, dram_pool, sb_pool, src_ap, shape):
    """AllReduce-add src_ap ([P,F] SBUF) across all 8 cores; returns SBUF tile."""
    bin_ = dram_pool.tile(list(shape), F32)
    bout = dram_pool.tile(list(shape), F32)
    nc.gpsimd.dma_start(bin_[:], src_ap)
    nc.gpsimd.collective_compute(
        "AllReduce", ALU.add,
        replica_groups=[list(range(N_CORES))],
        ins=[bin_.opt()], outs=[bout.opt()],
    )
    res = sb_pool.tile(list(shape), F32)
    nc.gpsimd.dma_start(res[:], bout[:])
    return res


def build(nc):
    # ---- DRAM I/O -------------------------------------------------------
    x_seq = nc.dram_tensor("x_seq", [T, BL, 3, 32, 32], F32, kind="ExternalInput")
    w1_dr = nc.dram_tensor("conv1_w", [64, 3, 3, 3], F32, kind="ExternalInput")
    g1_dr = nc.dram_tensor("bn1_g", [64], F32, kind="ExternalInput")
    b1_dr = nc.dram_tensor("bn1_b", [64], F32, kind="ExternalInput")
    w2_dr = nc.dram_tensor("conv2_w", [128, 64, 3, 3], F32, kind="ExternalInput")
    g2_dr = nc.dram_tensor("bn2_g", [128], F32, kind="ExternalInput")
    b2_dr = nc.dram_tensor("bn2_b", [128], F32, kind="ExternalInput")
    fc1_dr = nc.dram_tensor("fc1_w", [256, 8192], F32, kind="ExternalInput")
    fc2_dr = nc.dram_tensor("fc2_w", [10, 256], F32, kind="ExternalInput")
    fc2b_dr = nc.dram_tensor("fc2_b", [10], F32, kind="ExternalInput")
    out_dr = nc.dram_tensor("out", [T, BL, 10], F32, kind="ExternalOutput")

    with tile.TileContext(nc) as tc:
        import contextlib
        with contextlib.ExitStack() as ctx:
            dram = ctx.enter_context(tc.tile_pool(name="dram", bufs=1, space="DRAM"))
            persist = ctx.enter_context(tc.tile_pool(name="persist", bufs=1))

            # internal DRAM for layer outputs (pre-BN conv results, fp32)
            y1_dram = dram.tile([8, 16, 128, 512], F32)   # [pair, seg=(t,hh), part, col]
            y2_dram = dram.tile([T, BL, 128, 256], F32)   # [t, b, ch, hw]

            # persistent small tensors
            scale1 = persist.tile([128, T], F32)
            bias1 = persist.tile([128, T], F32)
            scale2 = persist.tile([128, T], F32)
            bias2 = persist.tile([128, T], F32)
            s1buf = persist.tile([128, 128], F32)
            s2buf = persist.tile([128, 128], F32)
            s1buf2 = persist.tile([128, 64], F32)
            s2buf2 = persist.tile([128, 64], F32)

            # conv1 weights: row r = dx*9+dy*3+ci, duplicated on 4 strips
            w1f = persist.tile([27, 64], F32)
            for dy in range(3):
                for dx in range(3):
                    r0 = dy * 9 + dx * 3
                    nc.sync.dma_start(
                        w1f[r0 : r0 + 3, :],
                        w1_dr.ap()[:, :, dy, dx].rearrange("c ci -> ci c"),
                    )
            w1_hi = persist.tile([128, 64], BF16)
            w1_lo = persist.tile([128, 64], BF16)
            nc.vector.tensor_copy(w1_hi[0:27], w1f[:])
            nc.vector.tensor_tensor(w1_lo[0:27], w1f[:], w1_hi[0:27], ALU.subtract)
            for j in (32, 64, 96):
                nc.sync.dma_start(w1_hi[j : j + 27], w1_hi[0:27])
                nc.sync.dma_start(w1_lo[j : j + 27], w1_lo[0:27])

            # conv2 weights per shift-group g=(dy,dx): [64ci, 9g, 128c], dup parity
            w2f = persist.tile([64, 9, 128], F32)
            nc.sync.dma_start(w2f[:], w2_dr.ap().rearrange("c ci dy dx -> ci (dy dx) c"))
            w2_hi = persist.tile([128, 9, 128], BF16)
            w2_lo = persist.tile([128, 9, 128], BF16)
            nc.vector.tensor_copy(w2_hi[0:64], w2f[:])
            nc.vector.tensor_tensor(w2_lo[0:64], w2f[:], w2_hi[0:64], ALU.subtract)
            nc.sync.dma_start(w2_hi[64:128], w2_hi[0:64])
            nc.sync.dma_start(w2_lo[64:128], w2_lo[0:64])

            # =============== STAGE A: conv1 + stats + store ===============
            # Padded per-channel planes staged in DRAM; each im2col row is a
            # contiguous shifted window of a plane -> few large DMAs.
            t_order = (0, 2, 4, 6, 1, 3, 5, 7)  # rotate im2col strips
            GUARD = 64
            PLANE = 128 * 1156  # (t b) frames of 34x34
            xflat_hi = dram.tile([3, GUARD + PLANE + GUARD], BF16)
            xflat_lo = dram.tile([3, GUARD + PLANE + GUARD], BF16)
            with tc.tile_pool(name="psumA", bufs=8, space="PSUM") as psum, \
                 tc.tile_pool(name="ysb", bufs=4) as ysb_pool, \
                 tc.tile_pool(name="sq", bufs=2) as sq_pool, \
                 tc.tile_pool(name="imc", bufs=1) as imc_pool:
                with tc.tile_pool(name="xstage", bufs=1) as xst:
                    # x staged as [(ci*34 + h+1), (t b), w+1] with zero pads
                    xpad = xst.tile([102, 128, 34], F32)
                    nc.vector.memset(xpad[:], 0.0)
                    for ci in range(3):
                        nc.sync.dma_start(
                            xpad[34 * ci + 1 : 34 * ci + 33, :, 1:33],
                            x_seq.ap()[:, :, ci].rearrange("t b h w -> h (t b) w"),
                        )
                    x_hi = xst.tile([102, 128, 34], BF16)
                    x_lo = xst.tile([102, 128, 34], BF16)
                    nc.vector.tensor_copy(x_hi[:], xpad[:])
                    nc.vector.tensor_tensor(x_lo[:], xpad[:], x_hi[:], ALU.subtract)
                    # stage padded planes out to DRAM (3 DMAs per tensor)
                    for ci in range(3):
                        for src_sb, dst_dr in ((x_hi, xflat_hi), (x_lo, xflat_lo)):
                            nc.sync.dma_start(
                                dst_dr[ci, GUARD : GUARD + PLANE]
                                    .rearrange("(tb h w) -> h tb w", h=34, w=34),
                                src_sb[34 * ci : 34 * ci + 34, :, :],
                            )

                # im2col strips: strip j (partitions 32j..32j+26) holds t in
                # {2j, 2j+1}; row r = dy*9 + dx*3 + ci; cols = padded frames.
                # Each (j,dy,dx) row-triple is one contiguous window per plane.
                SLEN = 2 * 16 * 1156  # 36992 cols per strip
                imc_hi = imc_pool.tile([128, SLEN], BF16)
                imc_lo = imc_pool.tile([128, SLEN], BF16)
                for j in range(4):
                    for dy in range(3):
                        for dx in range(3):
                            off = (GUARD + 2 * j * 16 * 1156
                                   + (dy - 1) * 34 + (dx - 1))
                            r0 = 32 * j + 9 * dy + 3 * dx
                            nc.sync.dma_start(
                                imc_hi[r0 : r0 + 3, :],
                                xflat_hi[:, off : off + SLEN],
                            )
                            nc.sync.dma_start(
                                imc_lo[r0 : r0 + 3, :],
                                xflat_lo[:, off : off + SLEN],
                            )

                ihi_v = imc_hi.rearrange("p (tb h w) -> p tb h w", h=34, w=34)
                ilo_v = imc_lo.rearrange("p (tb h w) -> p tb h w", h=34, w=34)
                for idx in range(128):
                    p = idx // 16
                    t = t_order[idx % 8]
                    hh = (idx // 8) % 2
                    j = t // 2
                    ps = psum.tile([128, 512], F32, tag="ps")
                    for half in range(2):
                        b = 2 * p + half
                        tbi = (t % 2) * 16 + b
                        h0 = hh * 16
                        args = [
                            (w1_hi, ihi_v), (w1_lo, ihi_v), (w1_hi, ilo_v),
                        ]
                        for k, (wt, im) in enumerate(args):
                            nc.tensor.matmul(
                                ps[64 * half : 64 * half + 64, :],
                                wt[32 * j : 32 * j + 27, :],
                                im[32 * j : 32 * j + 27, tbi,
                                   h0 + 1 : h0 + 17, 1:33],
                                start=(k == 0), stop=(k == 2),
                                tile_position=(32 * j, 64 * half),
                            )
                    y_sb = ysb_pool.tile([128, 512], F32)
                    nc.scalar.activation(y_sb[:], ps[:], AF.Identity,
                                         bias=0.0, scale=1.0,
                                         accum_out=s1buf[:, idx : idx + 1])
                    sq = sq_pool.tile([128, 512], F32)
                    nc.scalar.activation(sq[:], ps[:], AF.Square,
                                         bias=0.0, scale=1.0,
                                         accum_out=s2buf[:, idx : idx + 1])
                    seg = t * 2 + hh
                    nc.sync.dma_start(y1_dram[p, seg], y_sb[:])

            # =============== BN1 stats + allreduce ===============
            sums1 = persist.tile([128, 2], F32)
            nc.vector.tensor_reduce(sums1[:, 0:1], s1buf[:], mybir.AxisListType.X, ALU.add)
            nc.vector.tensor_reduce(sums1[:, 1:2], s2buf[:], mybir.AxisListType.X, ALU.add)
            g1 = _allreduce(nc, dram, persist, sums1[:], (128, 2))
            par1 = persist.tile([64, 2], F32)
            nc.sync.dma_start(par1[:], g1[64:128, :])
            tot1 = persist.tile([64, 2], F32)
            nc.vector.tensor_tensor(tot1[:], g1[0:64, :], par1[:], ALU.add)
            _stats_to_scale_bias(nc, persist, tot1, g1_dr, b1_dr,
                                 float(T * B_FULL * 32 * 32), 64, scale1, bias1)
            nc.sync.dma_start(scale1[64:128, :], scale1[0:64, :])
            nc.sync.dma_start(bias1[64:128, :], bias1[0:64, :])

            # fc weights + pooled2 (allocated after stage A frees its SBUF)
            fcpool = ctx.enter_context(tc.tile_pool(name="fcpool", bufs=1))
            fc1w = fcpool.tile([128, 256, 64], F32)  # [r, o, k] ; i = r*64+k
            nc.sync.dma_start(
                fc1w[:], fc1_dr.ap().rearrange("o (r k) -> r o k", r=128)
            )
            fc2w = fcpool.tile([128, 2, 10], F32)  # [r, m, o] ; i = m*128+r
            for m in range(2):
                nc.sync.dma_start(
                    fc2w[:, m, :],
                    fc2_dr.ap()[:, m * 128 : (m + 1) * 128].rearrange("o r -> r o"),
                )
            fc2b = fcpool.tile([10, 1], F32)
            nc.sync.dma_start(fc2b[:], fc2b_dr.ap()[:, None])
            pooled2 = fcpool.tile([128, 8192], F32)  # [(c), (t b hw)]

            # =============== STAGE B: LIF1 + pool ===============
            with tc.tile_pool(name="pooled1_pool", bufs=1) as pp1:
                pooled1 = pp1.tile([128, T, 8, 18, 18], BF16)
                nc.any.memset(pooled1[:], 0.0)
                with tc.tile_pool(name="stageB", bufs=3) as pB, \
                     tc.tile_pool(name="stateB", bufs=1) as stB:
                    for p in range(8):
                        st = [stB.tile([128, 1024], F32, tag=f"st{i}", name=f"stB{i}") for i in range(2)]
                        nc.any.memset(st[0][:], 0.0)
                        for t in range(T):
                            yc = pB.tile([128, 2, 512], F32, tag="yc", name="ycB")
                            nc.sync.dma_start(
                                yc[:],
                                y1_dram[p, 2 * t : 2 * t + 2].rearrange(
                                    "s part c -> part s c"),
                            )
                            a_new, a_old = st[(t + 1) % 2], st[t % 2]
                            nc.vector._custom_dve(
                                LIF_CHARGE, out=a_new[:], in0=a_old[:],
                                in1=yc.rearrange("p s c -> p (s c)"),
                                s0=scale1[:, t : t + 1], s1=bias1[:, t : t + 1],
                                imm2=float(2.0**t),
                            )
                            av = a_new.rearrange("p (h w) -> p h w", h=32)
                            hp = pB.tile([128, 32, 16], F32, tag="hp")
                            nc.vector._custom_dve(
                                SPIKE_HPOOL, out=hp[:],
                                in0=av[:, :, 0:32:2], in1=av[:, :, 1:32:2],
                                s0=float(2.0 ** (t + 1)),
                            )
                            nc.vector._custom_dve(
                                VPOOL_SCALE,
                                out=pooled1[:, t, p, 1:17, 1:17],
                                in0=hp[:, 0:32:2, :], in1=hp[:, 1:32:2, :],
                                s1=0.25,
                            )

                # =============== STAGE C: conv2 + stats + store ===============
                with tc.tile_pool(name="ysb2", bufs=4) as ysb2_pool, \
                     tc.tile_pool(name="psumC", bufs=8, space="PSUM") as psum, \
                     tc.tile_pool(name="sq2", bufs=2) as sq2_pool:
                    cidx = 0
                    for t in range(T):
                        for p0 in (0, 2, 4, 6):
                            for par in range(2):
                                ps = psum.tile([128, 512], F32, tag="ps")
                                k = 0
                                for wt in (w2_hi, w2_lo):
                                    for g in range(9):
                                        dy, dx = g // 3, g % 3
                                        rhs = pooled1[64 * par : 64 * par + 64, t,
                                                      p0 : p0 + 2,
                                                      dy : dy + 16, dx : dx + 16]
                                        nc.tensor.matmul(
                                            ps[:],
                                            wt[64 * par : 64 * par + 64, g, :],
                                            rhs,
                                            start=(k == 0), stop=(k == 17),
                                        )
                                        k += 1
                                y_sb = ysb2_pool.tile([128, 512], F32)
                                nc.scalar.activation(y_sb[:], ps[:], AF.Identity,
                                                     bias=0.0, scale=1.0,
                                                     accum_out=s1buf2[:, cidx : cidx + 1])
                                sq = sq2_pool.tile([128, 512], F32)
                                nc.scalar.activation(sq[:], ps[:], AF.Square,
                                                     bias=0.0, scale=1.0,
                                                     accum_out=s2buf2[:, cidx : cidx + 1])
                                cidx += 1
                                b0 = 2 * p0 + par
                                nc.sync.dma_start(
                                    y2_dram[t, b0 : b0 + 3 : 2].rearrange(
                                        "b p c -> p b c"),
                                    y_sb.rearrange("p (b c) -> p b c", b=2),
                                )

            # =============== BN2 stats + allreduce ===============
            sums2 = persist.tile([128, 2], F32)
            nc.vector.tensor_reduce(sums2[:, 0:1], s1buf2[:], mybir.AxisListType.X, ALU.add)
            nc.vector.tensor_reduce(sums2[:, 1:2], s2buf2[:], mybir.AxisListType.X, ALU.add)
            g2 = _allreduce(nc, dram, persist, sums2[:], (128, 2))
            _stats_to_scale_bias(nc, persist, g2, g2_dr, b2_dr,
                                 float(T * B_FULL * 16 * 16), 128, scale2, bias2)

            # =============== STAGE D: LIF2 + pool ===============
            with tc.tile_pool(name="stageD", bufs=2) as pD, \
                 tc.tile_pool(name="stateD", bufs=1) as stD:
                for bp in range(8):  # b-pairs
                    b0 = 2 * bp
                    yc = pD.tile([128, T, 2, 256], F32)
                    for t in range(T):
                        nc.sync.dma_start(
                            yc[:, t],
                            y2_dram[t, b0 : b0 + 2].rearrange("b p c -> p b c"),
                        )
                    ycv = yc.rearrange("p t b c -> p t (b c)")
                    st = [stD.tile([128, 512], F32, tag=f"std{i}", name=f"stD{i}") for i in range(2)]
                    nc.any.memset(st[0][:], 0.0)
                    for t in range(T):
                        a_new, a_old = st[(t + 1) % 2], st[t % 2]
                        nc.vector._custom_dve(
                            LIF_CHARGE, out=a_new[:], in0=a_old[:],
                            in1=ycv[:, t, :],
                            s0=scale2[:, t : t + 1], s1=bias2[:, t : t + 1],
                            imm2=float(2.0**t),
                        )
                        av = a_new.rearrange("p (bh w) -> p bh w", w=16)
                        hp = pD.tile([128, 32, 8], F32, tag="hp2")
                        nc.vector._custom_dve(
                            SPIKE_HPOOL, out=hp[:],
                            in0=av[:, :, 0:16:2], in1=av[:, :, 1:16:2],
                            s0=float(2.0 ** (t + 1)),
                        )
                        pout = pooled2[:, (t * 16 + b0) * 64 : (t * 16 + b0 + 2) * 64]
                        nc.vector._custom_dve(
                            VPOOL_SCALE,
                            out=pout.rearrange("p (bh w) -> p bh w", w=8),
                            in0=hp[:, 0:32:2, :], in1=hp[:, 1:32:2, :],
                            s1=0.25,
                        )

            # =============== STAGE E: fc1 + LIF + fc2 ===============
            p2v = pooled2.rearrange("p (tb k) -> p tb k", k=64)
            with tc.tile_pool(name="stageE", bufs=1) as pE, \
                 tc.tile_pool(name="psumE", bufs=2, space="PSUM") as psE:
                s_sb = pE.tile([128, 2, T, BL], F32)
                for m in range(2):
                    psf = psE.tile([128, 128], F32, tag="psf")
                    for k in range(64):
                        nc.tensor.matmul(
                            psf[:], fc1w[:, m * 128 : (m + 1) * 128, k],
                            p2v[:, :, k],
                            start=(k == 0), stop=(k == 63),
                        )
                    stf = [pE.tile([128, BL], F32, tag=f"stf{i}", name=f"stf{i}") for i in range(2)]
                    nc.any.memset(stf[0][:], 0.0)
                    for t in range(T):
                        a_new, a_old = stf[(t + 1) % 2], stf[t % 2]
                        nc.vector._custom_dve(
                            LIF_CHARGE, out=a_new[:], in0=a_old[:],
                            in1=psf[:, t * BL : (t + 1) * BL],
                            s0=float(2.0**t), s1=0.0, imm2=float(2.0**t),
                        )
                        nc.vector._custom_dve(
                            SPIKE_GE, out=s_sb[:, m, t, :], in0=a_new[:],
                            s0=float(2.0 ** (t + 1)),
                        )
                pso = psE.tile([10, 128], F32, tag="pso")
                sv = s_sb.rearrange("p m t b -> p m (t b)")
                nc.tensor.matmul(pso[:], fc2w[:, 0, :], sv[:, 0, :],
                                 start=True, stop=False)
                nc.tensor.matmul(pso[:], fc2w[:, 1, :], sv[:, 1, :],
                                 start=False, stop=True)
                out_sb = pE.tile([10, 128], F32)
                nc.scalar.activation(out_sb[:], pso[:], AF.Identity,
                                     bias=fc2b[:, 0:1], scale=1.0)
                nc.sync.dma_start(out_dr.ap().rearrange("t b o -> o (t b)"), out_sb[:])

    return nc


_CACHED = None


def _get_compiled():
    global _CACHED
    if _CACHED is None:
        nc = bacc.Bacc("TRN2", target_bir_lowering=False, debug=False,
                       num_devices=N_CORES)
        build(nc)
        nc.compile()
        _CACHED = nc
    return _CACHED


def kernel(**inputs) -> np.ndarray:
    nc = _get_compiled()
    np_in = {k: np.ascontiguousarray(np.asarray(v, dtype=np.float32))
             for k, v in inputs.items()}
    in_maps = []
    for i in range(N_CORES):
        m = dict(np_in)
        m["x_seq"] = np.ascontiguousarray(
            np_in["x_seq"][:, i * BL : (i + 1) * BL])
        in_maps.append(m)
    res = bass_utils.run_bass_kernel_spmd(nc, in_maps, core_ids=list(range(N_CORES)))
    return np.concatenate([res.results[i]["out"] for i in range(N_CORES)], axis=1)


if __name__ == "__main__":
    nc = _get_compiled()
    print("compiled OK")


# revision 15
# speedup vs baseline: 1.2249x; 1.2249x over previous
"""Trainium2 Bass kernel for nn_CifarBaselineSNN.

conv1(3->64,3x3,p1) -> BN -> LIF -> avgpool2
conv2(64->128,3x3,p1) -> BN -> LIF -> avgpool2
fc1(8192->256) -> LIF -> fc2(256->10)+b
T=8, B=128. Data-parallel over B across 8 NeuronCores (16 samples/core);
BN statistics are global over the batch -> 2 small AllReduces.

Numerics: convolutions run as bf16 hi/lo weight-split matmuls accumulating in
fp32 PSUM (inputs to conv2 are pooled spikes, exactly representable in bf16;
conv1 inputs are hi/lo split too). LIF state uses the 2^t scaling trick so one
fused DVE op does decay+charge+reset per step.
"""

import sys
import os

for _p in ("/opt/trn_rl_repo", "/root/.axon_site/_ro/trn_rl_repo"):
    if os.path.isdir(_p) and _p not in sys.path:
        sys.path.append(_p)

import numpy as np

import concourse.bass as bass
import concourse.mybir as mybir
import concourse.tile as tile
from concourse import bacc
from concourse import bass_utils
from concourse import dve_ops as _dops
from concourse.dve_uop import DveOpSpec
from concourse.dve_spec import (
    Spec, Src0, Src1, C0, C1, C2, Zero, select, lower, _has_src1,
)

F32 = mybir.dt.float32
BF16 = mybir.dt.bfloat16
AF = mybir.ActivationFunctionType
ALU = mybir.AluOpType

T = 8
B_FULL = 128
N_CORES = 8
BL = B_FULL // N_CORES  # 16 samples per core
EPS = 1e-5


# --------------------------------------------------------------------------
# Custom DVE ops (fused LIF pieces)
# --------------------------------------------------------------------------

def _register_op(name, spec, ref):
    for op in _dops.OPS:
        if op.name == name:
            return op
    spec = Spec(body=spec.body, accum=spec.accum, accum_init=spec.accum_init,
                reference=ref)
    shas = {}
    for ver in ("v3", "v4"):
        s = DveOpSpec(name=name, opcode=0, uops=lower(spec, ver=ver),
                      rd1_en=_has_src1(spec))
        shas[ver] = s.sha(ver)
    op = _dops.DveOp(name, spec, subdim=False, uops_sha=shas)
    _dops.OPS.append(op)
    _dops.CUSTOM_DVE_SPECS[name] = spec
    _dops._SUB_OPCODE_FOR_NAME[name] = max(_dops._SUB_OPCODE_FOR_NAME.values()) + 1
    return op


# A_t = (A_{t-1} if A_{t-1} < theta_{t-1} else 0) + y*scale + bias
LIF_CHARGE = _register_op(
    "LIF_CHARGE_SNN",
    Spec(body=select(Src0 >= C2, Zero, Src0) + Src1 * C0 + C1),
    lambda in0, in1, s0, s1, imm2: np.where(in0 >= imm2, 0.0, in0) + in1 * s0 + s1,
)

# spike counts over horizontal pairs: (a>=th) + (b>=th)   (values 0/1/2)
SPIKE_HPOOL = _register_op(
    "SPIKE_HPOOL_SNN",
    Spec(body=(Src0 >= C0) + (Src1 >= C0)),
    lambda in0, in1, s0, s1, imm2: (in0 >= s0).astype(np.float32)
    + (in1 >= s0).astype(np.float32),
)

# pooled = (hpA + hpB) * 0.25
VPOOL_SCALE = _register_op(
    "VPOOL_SCALE_SNN",
    Spec(body=(Src0 + Src1) * C1),
    lambda in0, in1, s0, s1, imm2: (in0 + in1) * s1,
)

# plain spike: (a >= th)
SPIKE_GE = _register_op(
    "SPIKE_GE_SNN",
    Spec(body=(Src0 >= C0) + Zero),
    lambda in0, s0, s1, imm2: (in0 >= s0).astype(np.float32),
)


# --------------------------------------------------------------------------
# Kernel build
# --------------------------------------------------------------------------

def _stats_to_scale_bias(nc, pool, tot, g_dr, b_dr, n_count, nch, out_scale, out_bias):
    """tot: [nch,2] SBUF (sum, sumsq). Writes out_scale/out_bias [nch,8]:
    scale[:,t] = gamma*rstd*2^t ; bias[:,t] = (beta - mu*gamma*rstd)*2^t."""
    mu = pool.tile([nch, 1], F32)
    nc.vector.tensor_scalar_mul(mu[:], tot[:, 0:1], 1.0 / n_count)
    e2 = pool.tile([nch, 1], F32)
    nc.vector.tensor_scalar_mul(e2[:], tot[:, 1:2], 1.0 / n_count)
    var = pool.tile([nch, 1], F32)
    nc.vector.tensor_tensor(var[:], mu[:], mu[:], ALU.mult)
    nc.vector.tensor_tensor(var[:], e2[:], var[:], ALU.subtract)
    nc.vector.tensor_scalar_add(var[:], var[:], float(EPS))
    std = pool.tile([nch, 1], F32)
    nc.scalar.activation(std[:], var[:], AF.Sqrt, bias=0.0, scale=1.0)
    rstd = pool.tile([nch, 1], F32)
    nc.vector.reciprocal(rstd[:], std[:])
    gam = pool.tile([nch, 1], F32)
    nc.sync.dma_start(gam[:], g_dr.ap()[:, None])
    bet = pool.tile([nch, 1], F32)
    nc.sync.dma_start(bet[:], b_dr.ap()[:, None])
    gr = pool.tile([nch, 1], F32)
    nc.vector.tensor_tensor(gr[:], gam[:], rstd[:], ALU.mult)
    bb = pool.tile([nch, 1], F32)  # beta - mu*gr
    nc.vector.tensor_tensor(bb[:], mu[:], gr[:], ALU.mult)
    nc.vector.tensor_tensor(bb[:], bet[:], bb[:], ALU.subtract)
    for t in range(T):
        nc.vector.tensor_scalar_mul(out_scale[:nch, t : t + 1], gr[:], float(2.0**t))
        nc.vector.tensor_scalar_mul(out_bias[:nch, t : t + 1], bb[:], float(2.0**t))


def _allreduce(nc, dram_pool, sb_pool, src_ap, shape):
    """AllReduce-add src_ap ([P,F] SBUF) across all 8 cores; returns SBUF tile."""
    bin_ = dram_pool.tile(list(shape), F32)
    bout = dram_pool.tile(list(shape), F32)
    nc.gpsimd.dma_start(bin_[:], src_ap)
    nc.gpsimd.collective_compute(
        "AllReduce", ALU.add,
        replica_groups=[list(range(N_CORES))],
        ins=[bin_.opt()], outs=[bout.opt()],
    )
    res = sb_pool.tile(list(shape), F32)
    nc.gpsimd.dma_start(res[:], bout[:])
    return res


def build(nc):
    # ---- DRAM I/O -------------------------------------------------------
    x_seq = nc.dram_tensor("x_seq", [T, BL, 3, 32, 32], F32, kind="ExternalInput")
    w1_dr = nc.dram_tensor("conv1_w", [64, 3, 3, 3], F32, kind="ExternalInput")
    g1_dr = nc.dram_tensor("bn1_g", [64], F32, kind="ExternalInput")
    b1_dr = nc.dram_tensor("bn1_b", [64], F32, kind="ExternalInput")
    w2_dr = nc.dram_tensor("conv2_w", [128, 64, 3, 3], F32, kind="ExternalInput")
    g2_dr = nc.dram_tensor("bn2_g", [128], F32, kind="ExternalInput")
    b2_dr = nc.dram_tensor("bn2_b", [128], F32, kind="ExternalInput")
    fc1_dr = nc.dram_tensor("fc1_w", [256, 8192], F32, kind="ExternalInput")
    fc2_dr = nc.dram_tensor("fc2_w", [10, 256], F32, kind="ExternalInput")
    fc2b_dr = nc.dram_tensor("fc2_b", [10], F32, kind="ExternalInput")
    out_dr = nc.dram_tensor("out", [T, BL, 10], F32, kind="ExternalOutput")

    with tile.TileContext(nc) as tc:
        import contextlib
        with contextlib.ExitStack() as ctx:
            dram = ctx.enter_context(tc.tile_pool(name="dram", bufs=1, space="DRAM"))
            persist = ctx.enter_context(tc.tile_pool(name="persist", bufs=1))

            # internal DRAM for layer outputs (pre-BN conv results, fp32)
            y1_dram = dram.tile([8, 16, 128, 512], F32)   # [pair, seg=(t,hh), part, col]
            y2_dram = dram.tile([T, BL, 128, 256], F32)   # [t, b, ch, hw]

            # persistent small tensors
            scale1 = persist.tile([128, T], F32)
            bias1 = persist.tile([128, T], F32)
            scale2 = persist.tile([128, T], F32)
            bias2 = persist.tile([128, T], F32)
            s1buf = persist.tile([128, 128], F32)
            s2buf = persist.tile([128, 128], F32)
            s1buf2 = persist.tile([128, 64], F32)
            s2buf2 = persist.tile([128, 64], F32)

            # conv1 weights: row r = dx*9+dy*3+ci, duplicated on 4 strips
            w1f = persist.tile([27, 64], F32)
            for dy in range(3):
                for dx in range(3):
                    r0 = dy * 9 + dx * 3
                    nc.sync.dma_start(
                        w1f[r0 : r0 + 3, :],
                        w1_dr.ap()[:, :, dy, dx].rearrange("c ci -> ci c"),
                    )
            w1_hi = persist.tile([128, 64], BF16)
            w1_lo = persist.tile([128, 64], BF16)
            nc.vector.tensor_copy(w1_hi[0:27], w1f[:])
            nc.vector.tensor_tensor(w1_lo[0:27], w1f[:], w1_hi[0:27], ALU.subtract)
            for j in (32, 64, 96):
                nc.sync.dma_start(w1_hi[j : j + 27], w1_hi[0:27])
                nc.sync.dma_start(w1_lo[j : j + 27], w1_lo[0:27])

            # conv2 weights per shift-group g=(dy,dx): [64ci, 9g, 128c], dup parity
            w2f = persist.tile([64, 9, 128], F32)
            nc.sync.dma_start(w2f[:], w2_dr.ap().rearrange("c ci dy dx -> ci (dy dx) c"))
            w2_hi = persist.tile([128, 9, 128], BF16)
            w2_lo = persist.tile([128, 9, 128], BF16)
            nc.vector.tensor_copy(w2_hi[0:64], w2f[:])
            nc.vector.tensor_tensor(w2_lo[0:64], w2f[:], w2_hi[0:64], ALU.subtract)
            nc.sync.dma_start(w2_hi[64:128], w2_hi[0:64])
            nc.sync.dma_start(w2_lo[64:128], w2_lo[0:64])

            # =============== STAGE A: conv1 + stats + store ===============
            # Padded per-channel planes staged in DRAM; each im2col row is a
            # contiguous shifted window of a plane -> few large DMAs.
            t_order = (0, 2, 4, 6, 1, 3, 5, 7)  # rotate im2col strips
            GUARD = 64
            PLANE = 128 * 1156  # (t b) frames of 34x34
            xflat_hi = dram.tile([3, GUARD + PLANE + GUARD], BF16)
            xflat_lo = dram.tile([3, GUARD + PLANE + GUARD], BF16)
            with tc.tile_pool(name="psumA", bufs=8, space="PSUM") as psum, \
                 tc.tile_pool(name="ysb", bufs=4) as ysb_pool, \
                 tc.tile_pool(name="sq", bufs=2) as sq_pool, \
                 tc.tile_pool(name="imc", bufs=1) as imc_pool:
                with tc.tile_pool(name="xstage", bufs=1) as xst:
                    # x staged with (t b) on partitions: padded frames are
                    # contiguous per partition -> 2.3KB DMA runs to DRAM.
                    xpadF = xst.tile([128, 3, 34, 34], F32)
                    nc.vector.memset(xpadF[:], 0.0)
                    for ci in range(3):
                        nc.sync.dma_start(
                            xpadF[:, ci, 1:33, 1:33],
                            x_seq.ap()[:, :, ci].rearrange("t b h w -> (t b) h w"),
                        )
                    xpad_flat = xpadF.rearrange("p c h w -> p (c h w)")
                    x_hiF = xst.tile([128, 3, 34, 34], BF16)
                    x_loF = xst.tile([128, 3, 34, 34], BF16)
                    xhi_flat = x_hiF.rearrange("p c h w -> p (c h w)")
                    xlo_flat = x_loF.rearrange("p c h w -> p (c h w)")
                    nc.vector.tensor_copy(xhi_flat[:], xpad_flat[:])
                    nc.vector.tensor_tensor(xlo_flat[:], xpad_flat[:],
                                            xhi_flat[:], ALU.subtract)
                    for ci in range(3):
                        for src_sb, dst_dr in ((x_hiF, xflat_hi), (x_loF, xflat_lo)):
                            nc.sync.dma_start(
                                dst_dr[ci, GUARD : GUARD + PLANE]
                                    .rearrange("(tb f) -> tb f", f=1156),
                                src_sb[:, ci].rearrange("p h w -> p (h w)"),
                            )

                # im2col strips: strip j (partitions 32j..32j+26) holds t in
                # {2j, 2j+1}; row r = dy*9 + dx*3 + ci; cols = padded frames.
                # Each (j,dy,dx) row-triple is one contiguous window per plane.
                SLEN = 2 * 16 * 1156  # 36992 cols per strip
                imc_hi = imc_pool.tile([128, SLEN], BF16)
                imc_lo = imc_pool.tile([128, SLEN], BF16)
                _qs = (nc.sync, nc.scalar, nc.gpsimd)
                _qi = 0
                for j in range(4):
                    for dy in range(3):
                        for dx in range(3):
                            off = (GUARD + 2 * j * 16 * 1156
                                   + (dy - 1) * 34 + (dx - 1))
                            r0 = 32 * j + 9 * dy + 3 * dx
                            _qs[_qi % 3].dma_start(
                                imc_hi[r0 : r0 + 3, :],
                                xflat_hi[:, off : off + SLEN],
                            )
                            _qs[(_qi + 1) % 3].dma_start(
                                imc_lo[r0 : r0 + 3, :],
                                xflat_lo[:, off : off + SLEN],
                            )
                            _qi += 2

                ihi_v = imc_hi.rearrange("p (tb h w) -> p tb h w", h=34, w=34)
                ilo_v = imc_lo.rearrange("p (tb h w) -> p tb h w", h=34, w=34)
                for idx in range(128):
                    p = idx // 16
                    t = t_order[idx % 8]
                    hh = (idx // 8) % 2
                    j = t // 2
                    ps = psum.tile([128, 512], F32, tag="ps")
                    for half in range(2):
                        b = 2 * p + half
                        tbi = (t % 2) * 16 + b
                        h0 = hh * 16
                        args = [
                            (w1_hi, ihi_v), (w1_lo, ihi_v), (w1_hi, ilo_v),
                        ]
                        for k, (wt, im) in enumerate(args):
                            nc.tensor.matmul(
                                ps[64 * half : 64 * half + 64, :],
                                wt[32 * j : 32 * j + 27, :],
                                im[32 * j : 32 * j + 27, tbi,
                                   h0 + 1 : h0 + 17, 1:33],
                                start=(k == 0), stop=(k == 2),
                                tile_position=(32 * j, 64 * half),
                            )
                    y_sb = ysb_pool.tile([128, 512], F32)
                    nc.scalar.activation(y_sb[:], ps[:], AF.Identity,
                                         bias=0.0, scale=1.0,
                                         accum_out=s1buf[:, idx : idx + 1])
                    sq = sq_pool.tile([128, 512], F32)
                    nc.scalar.activation(sq[:], ps[:], AF.Square,
                                         bias=0.0, scale=1.0,
                                         accum_out=s2buf[:, idx : idx + 1])
                    seg = t * 2 + hh
                    (nc.sync if idx % 2 == 0 else nc.scalar).dma_start(
                        y1_dram[p, seg], y_sb[:])

            # =============== BN1 stats + allreduce ===============
            sums1 = persist.tile([128, 2], F32)
            nc.vector.tensor_reduce(sums1[:, 0:1], s1buf[:], mybir.AxisListType.X, ALU.add)
            nc.vector.tensor_reduce(sums1[:, 1:2], s2buf[:], mybir.AxisListType.X, ALU.add)
            g1 = _allreduce(nc, dram, persist, sums1[:], (128, 2))
            par1 = persist.tile([64, 2], F32)
            nc.sync.dma_start(par1[:], g1[64:128, :])
            tot1 = persist.tile([64, 2], F32)
            nc.vector.tensor_tensor(tot1[:], g1[0:64, :], par1[:], ALU.add)
            _stats_to_scale_bias(nc, persist, tot1, g1_dr, b1_dr,
                                 float(T * B_FULL * 32 * 32), 64, scale1, bias1)
            nc.sync.dma_start(scale1[64:128, :], scale1[0:64, :])
            nc.sync.dma_start(bias1[64:128, :], bias1[0:64, :])

            # fc weights + pooled2 (allocated after stage A frees its SBUF)
            fcpool = ctx.enter_context(tc.tile_pool(name="fcpool", bufs=1))
            fc1w = fcpool.tile([128, 256, 64], F32)  # [r, o, k] ; i = r*64+k
            nc.sync.dma_start(
                fc1w[:], fc1_dr.ap().rearrange("o (r k) -> r o k", r=128)
            )
            fc2w = fcpool.tile([128, 2, 10], F32)  # [r, m, o] ; i = m*128+r
            for m in range(2):
                nc.sync.dma_start(
                    fc2w[:, m, :],
                    fc2_dr.ap()[:, m * 128 : (m + 1) * 128].rearrange("o r -> r o"),
                )
            fc2b = fcpool.tile([10, 1], F32)
            nc.sync.dma_start(fc2b[:], fc2b_dr.ap()[:, None])
            pooled2 = fcpool.tile([128, 8192], F32)  # [(c), (t b hw)]

            # =============== STAGE B: LIF1 + pool ===============
            with tc.tile_pool(name="pooled1_pool", bufs=1) as pp1:
                pooled1 = pp1.tile([128, T, 8, 18, 18], BF16)
                nc.any.memset(pooled1[:], 0.0)
                with tc.tile_pool(name="stageB", bufs=3) as pB, \
                     tc.tile_pool(name="stateB", bufs=1) as stB:
                    for p in range(8):
                        st = [stB.tile([128, 1024], F32, tag=f"st{i}", name=f"stB{i}") for i in range(2)]
                        nc.any.memset(st[0][:], 0.0)
                        for t in range(T):
                            yc = pB.tile([128, 2, 512], F32, tag="yc", name="ycB")
                            nc.sync.dma_start(
                                yc[:],
                                y1_dram[p, 2 * t : 2 * t + 2].rearrange(
                                    "s part c -> part s c"),
                            )
                            a_new, a_old = st[(t + 1) % 2], st[t % 2]
                            nc.vector._custom_dve(
                                LIF_CHARGE, out=a_new[:], in0=a_old[:],
                                in1=yc.rearrange("p s c -> p (s c)"),
                                s0=scale1[:, t : t + 1], s1=bias1[:, t : t + 1],
                                imm2=float(2.0**t),
                            )
                            av = a_new.rearrange("p (h w) -> p h w", h=32)
                            hp = pB.tile([128, 32, 16], F32, tag="hp")
                            nc.vector._custom_dve(
                                SPIKE_HPOOL, out=hp[:],
                                in0=av[:, :, 0:32:2], in1=av[:, :, 1:32:2],
                                s0=float(2.0 ** (t + 1)),
                            )
                            nc.vector._custom_dve(
                                VPOOL_SCALE,
                                out=pooled1[:, t, p, 1:17, 1:17],
                                in0=hp[:, 0:32:2, :], in1=hp[:, 1:32:2, :],
                                s1=0.25,
                            )

                # =============== STAGE C: conv2 + stats + store ===============
                with tc.tile_pool(name="ysb2", bufs=4) as ysb2_pool, \
                     tc.tile_pool(name="psumC", bufs=8, space="PSUM") as psum, \
                     tc.tile_pool(name="sq2", bufs=2) as sq2_pool:
                    cidx = 0
                    for t in range(T):
                        for p0 in (0, 2, 4, 6):
                            for par in range(2):
                                ps = psum.tile([128, 512], F32, tag="ps")
                                k = 0
                                for wt in (w2_hi, w2_lo):
                                    for g in range(9):
                                        dy, dx = g // 3, g % 3
                                        rhs = pooled1[64 * par : 64 * par + 64, t,
                                                      p0 : p0 + 2,
                                                      dy : dy + 16, dx : dx + 16]
                                        nc.tensor.matmul(
                                            ps[:],
                                            wt[64 * par : 64 * par + 64, g, :],
                                            rhs,
                                            start=(k == 0), stop=(k == 17),
                                        )
                                        k += 1
                                y_sb = ysb2_pool.tile([128, 512], F32)
                                nc.scalar.activation(y_sb[:], ps[:], AF.Identity,
                                                     bias=0.0, scale=1.0,
                                                     accum_out=s1buf2[:, cidx : cidx + 1])
                                sq = sq2_pool.tile([128, 512], F32)
                                nc.scalar.activation(sq[:], ps[:], AF.Square,
                                                     bias=0.0, scale=1.0,
                                                     accum_out=s2buf2[:, cidx : cidx + 1])
                                cidx += 1
                                b0 = 2 * p0 + par
                                nc.sync.dma_start(
                                    y2_dram[t, b0 : b0 + 3 : 2].rearrange(
                                        "b p c -> p b c"),
                                    y_sb.rearrange("p (b c) -> p b c", b=2),
                                )

            # =============== BN2 stats + allreduce ===============
            sums2 = persist.tile([128, 2], F32)
            nc.vector.tensor_reduce(sums2[:, 0:1], s1buf2[:], mybir.AxisListType.X, ALU.add)
            nc.vector.tensor_reduce(sums2[:, 1:2], s2buf2[:], mybir.AxisListType.X, ALU.add)
            g2 = _allreduce(nc, dram, persist, sums2[:], (128, 2))
            _stats_to_scale_bias(nc, persist, g2, g2_dr, b2_dr,
                                 float(T * B_FULL * 16 * 16), 128, scale2, bias2)

            # =============== STAGE D: LIF2 + pool ===============
            with tc.tile_pool(name="stageD", bufs=2) as pD, \
                 tc.tile_pool(name="stateD", bufs=1) as stD:
                for bp in range(8):  # b-pairs
                    b0 = 2 * bp
                    yc = pD.tile([128, T, 2, 256], F32)
                    for t in range(T):
                        nc.sync.dma_start(
                            yc[:, t],
                            y2_dram[t, b0 : b0 + 2].rearrange("b p c -> p b c"),
                        )
                    ycv = yc.rearrange("p t b c -> p t (b c)")
                    st = [stD.tile([128, 512], F32, tag=f"std{i}", name=f"stD{i}") for i in range(2)]
                    nc.any.memset(st[0][:], 0.0)
                    for t in range(T):
                        a_new, a_old = st[(t + 1) % 2], st[t % 2]
                        nc.vector._custom_dve(
                            LIF_CHARGE, out=a_new[:], in0=a_old[:],
                            in1=ycv[:, t, :],
                            s0=scale2[:, t : t + 1], s1=bias2[:, t : t + 1],
                            imm2=float(2.0**t),
                        )
                        av = a_new.rearrange("p (bh w) -> p bh w", w=16)
                        hp = pD.tile([128, 32, 8], F32, tag="hp2")
                        nc.vector._custom_dve(
                            SPIKE_HPOOL, out=hp[:],
                            in0=av[:, :, 0:16:2], in1=av[:, :, 1:16:2],
                            s0=float(2.0 ** (t + 1)),
                        )
                        pout = pooled2[:, (t * 16 + b0) * 64 : (t * 16 + b0 + 2) * 64]
                        nc.vector._custom_dve(
                            VPOOL_SCALE,
                            out=pout.rearrange("p (bh w) -> p bh w", w=8),
                            in0=hp[:, 0:32:2, :], in1=hp[:, 1:32:2, :],
                            s1=0.25,
                        )

            # =============== STAGE E: fc1 + LIF + fc2 ===============
            p2v = pooled2.rearrange("p (tb k) -> p tb k", k=64)
            with tc.tile_pool(name="stageE", bufs=1) as pE, \
                 tc.tile_pool(name="psumE", bufs=2, space="PSUM") as psE:
                s_sb = pE.tile([128, 2, T, BL], F32)
                for m in range(2):
                    psf = psE.tile([128, 128], F32, tag="psf")
                    for k in range(64):
                        nc.tensor.matmul(
                            psf[:], fc1w[:, m * 128 : (m + 1) * 128, k],
                            p2v[:, :, k],
                            start=(k == 0), stop=(k == 63),
                        )
                    stf = [pE.tile([128, BL], F32, tag=f"stf{i}", name=f"stf{i}") for i in range(2)]
                    nc.any.memset(stf[0][:], 0.0)
                    for t in range(T):
                        a_new, a_old = stf[(t + 1) % 2], stf[t % 2]
                        nc.vector._custom_dve(
                            LIF_CHARGE, out=a_new[:], in0=a_old[:],
                            in1=psf[:, t * BL : (t + 1) * BL],
                            s0=float(2.0**t), s1=0.0, imm2=float(2.0**t),
                        )
                        nc.vector._custom_dve(
                            SPIKE_GE, out=s_sb[:, m, t, :], in0=a_new[:],
                            s0=float(2.0 ** (t + 1)),
                        )
                pso = psE.tile([10, 128], F32, tag="pso")
                sv = s_sb.rearrange("p m t b -> p m (t b)")
                nc.tensor.matmul(pso[:], fc2w[:, 0, :], sv[:, 0, :],
                                 start=True, stop=False)
                nc.tensor.matmul(pso[:], fc2w[:, 1, :], sv[:, 1, :],
                                 start=False, stop=True)
                out_sb = pE.tile([10, 128], F32)
                nc.scalar.activation(out_sb[:], pso[:], AF.Identity,
                                     bias=fc2b[:, 0:1], scale=1.0)
                nc.sync.dma_start(out_dr.ap().rearrange("t b o -> o (t b)"), out_sb[:])

    return nc


_CACHED = None


def _get_compiled():
    global _CACHED
    if _CACHED is None:
        nc = bacc.Bacc("TRN2", target_bir_lowering=False, debug=False,
                       num_devices=N_CORES)
        build(nc)
        nc.compile()
        _CACHED = nc
    return _CACHED


def kernel(**inputs) -> np.ndarray:
    nc = _get_compiled()
    np_in = {k: np.ascontiguousarray(np.asarray(v, dtype=np.float32))
             for k, v in inputs.items()}
    in_maps = []
    for i in range(N_CORES):
        m = dict(np_in)
        m["x_seq"] = np.ascontiguousarray(
            np_in["x_seq"][:, i * BL : (i + 1) * BL])
        in_maps.append(m)
    res = bass_utils.run_bass_kernel_spmd(nc, in_maps, core_ids=list(range(N_CORES)))
    return np.concatenate([res.results[i]["out"] for i in range(N_CORES)], axis=1)


if __name__ == "__main__":
    nc = _get_compiled()
    print("compiled OK")


# revision 16
# speedup vs baseline: 1.2570x; 1.0263x over previous
"""Trainium2 Bass kernel for nn_CifarBaselineSNN.

conv1(3->64,3x3,p1) -> BN -> LIF -> avgpool2
conv2(64->128,3x3,p1) -> BN -> LIF -> avgpool2
fc1(8192->256) -> LIF -> fc2(256->10)+b
T=8, B=128. Data-parallel over B across 8 NeuronCores (16 samples/core);
BN statistics are global over the batch -> 2 small AllReduces.

Numerics: convolutions run as bf16 hi/lo weight-split matmuls accumulating in
fp32 PSUM (inputs to conv2 are pooled spikes, exactly representable in bf16;
conv1 inputs are hi/lo split too). LIF state uses the 2^t scaling trick so one
fused DVE op does decay+charge+reset per step.
"""

import sys
import os

for _p in ("/opt/trn_rl_repo", "/root/.axon_site/_ro/trn_rl_repo"):
    if os.path.isdir(_p) and _p not in sys.path:
        sys.path.append(_p)

import numpy as np

import concourse.bass as bass
import concourse.mybir as mybir
import concourse.tile as tile
from concourse import bacc
from concourse import bass_utils
from concourse import dve_ops as _dops
from concourse.dve_uop import DveOpSpec
from concourse.dve_spec import (
    Spec, Src0, Src1, C0, C1, C2, Zero, select, lower, _has_src1,
)

F32 = mybir.dt.float32
BF16 = mybir.dt.bfloat16
AF = mybir.ActivationFunctionType
ALU = mybir.AluOpType

T = 8
B_FULL = 128
N_CORES = 8
BL = B_FULL // N_CORES  # 16 samples per core
EPS = 1e-5


# --------------------------------------------------------------------------
# Custom DVE ops (fused LIF pieces)
# --------------------------------------------------------------------------

def _register_op(name, spec, ref):
    for op in _dops.OPS:
        if op.name == name:
            return op
    spec = Spec(body=spec.body, accum=spec.accum, accum_init=spec.accum_init,
                reference=ref)
    shas = {}
    for ver in ("v3", "v4"):
        s = DveOpSpec(name=name, opcode=0, uops=lower(spec, ver=ver),
                      rd1_en=_has_src1(spec))
        shas[ver] = s.sha(ver)
    op = _dops.DveOp(name, spec, subdim=False, uops_sha=shas)
    _dops.OPS.append(op)
    _dops.CUSTOM_DVE_SPECS[name] = spec
    _dops._SUB_OPCODE_FOR_NAME[name] = max(_dops._SUB_OPCODE_FOR_NAME.values()) + 1
    return op


# A_t = (A_{t-1} if A_{t-1} < theta_{t-1} else 0) + y*scale + bias
LIF_CHARGE = _register_op(
    "LIF_CHARGE_SNN",
    Spec(body=select(Src0 >= C2, Zero, Src0) + Src1 * C0 + C1),
    lambda in0, in1, s0, s1, imm2: np.where(in0 >= imm2, 0.0, in0) + in1 * s0 + s1,
)

# spike counts over horizontal pairs: (a>=th) + (b>=th)   (values 0/1/2)
SPIKE_HPOOL = _register_op(
    "SPIKE_HPOOL_SNN",
    Spec(body=(Src0 >= C0) + (Src1 >= C0)),
    lambda in0, in1, s0, s1, imm2: (in0 >= s0).astype(np.float32)
    + (in1 >= s0).astype(np.float32),
)

# pooled = (hpA + hpB) * 0.25
VPOOL_SCALE = _register_op(
    "VPOOL_SCALE_SNN",
    Spec(body=(Src0 + Src1) * C1),
    lambda in0, in1, s0, s1, imm2: (in0 + in1) * s1,
)

# plain spike: (a >= th)
SPIKE_GE = _register_op(
    "SPIKE_GE_SNN",
    Spec(body=(Src0 >= C0) + Zero),
    lambda in0, s0, s1, imm2: (in0 >= s0).astype(np.float32),
)


# --------------------------------------------------------------------------
# Kernel build
# --------------------------------------------------------------------------

def _stats_to_scale_bias(nc, pool, tot, g_dr, b_dr, n_count, nch, out_scale, out_bias):
    """tot: [nch,2] SBUF (sum, sumsq). Writes out_scale/out_bias [nch,8]:
    scale[:,t] = gamma*rstd*2^t ; bias[:,t] = (beta - mu*gamma*rstd)*2^t."""
    mu = pool.tile([nch, 1], F32)
    nc.vector.tensor_scalar_mul(mu[:], tot[:, 0:1], 1.0 / n_count)
    e2 = pool.tile([nch, 1], F32)
    nc.vector.tensor_scalar_mul(e2[:], tot[:, 1:2], 1.0 / n_count)
    var = pool.tile([nch, 1], F32)
    nc.vector.tensor_tensor(var[:], mu[:], mu[:], ALU.mult)
    nc.vector.tensor_tensor(var[:], e2[:], var[:], ALU.subtract)
    nc.vector.tensor_scalar_add(var[:], var[:], float(EPS))
    std = pool.tile([nch, 1], F32)
    nc.scalar.activation(std[:], var[:], AF.Sqrt, bias=0.0, scale=1.0)
    rstd = pool.tile([nch, 1], F32)
    nc.vector.reciprocal(rstd[:], std[:])
    gam = pool.tile([nch, 1], F32)
    nc.sync.dma_start(gam[:], g_dr.ap()[:, None])
    bet = pool.tile([nch, 1], F32)
    nc.sync.dma_start(bet[:], b_dr.ap()[:, None])
    gr = pool.tile([nch, 1], F32)
    nc.vector.tensor_tensor(gr[:], gam[:], rstd[:], ALU.mult)
    bb = pool.tile([nch, 1], F32)  # beta - mu*gr
    nc.vector.tensor_tensor(bb[:], mu[:], gr[:], ALU.mult)
    nc.vector.tensor_tensor(bb[:], bet[:], bb[:], ALU.subtract)
    for t in range(T):
        nc.vector.tensor_scalar_mul(out_scale[:nch, t : t + 1], gr[:], float(2.0**t))
        nc.vector.tensor_scalar_mul(out_bias[:nch, t : t + 1], bb[:], float(2.0**t))


def _allreduce(nc, dram_pool, sb_pool, src_ap, shape):
    """AllReduce-add src_ap ([P,F] SBUF) across all 8 cores; returns SBUF tile."""
    bin_ = dram_pool.tile(list(shape), F32)
    bout = dram_pool.tile(list(shape), F32)
    nc.gpsimd.dma_start(bin_[:], src_ap)
    nc.gpsimd.collective_compute(
        "AllReduce", ALU.add,
        replica_groups=[list(range(N_CORES))],
        ins=[bin_.opt()], outs=[bout.opt()],
    )
    res = sb_pool.tile(list(shape), F32)
    nc.gpsimd.dma_start(res[:], bout[:])
    return res


def build(nc):
    # ---- DRAM I/O -------------------------------------------------------
    x_seq = nc.dram_tensor("x_seq", [T, BL, 3, 32, 32], F32, kind="ExternalInput")
    w1_dr = nc.dram_tensor("conv1_w", [64, 3, 3, 3], F32, kind="ExternalInput")
    g1_dr = nc.dram_tensor("bn1_g", [64], F32, kind="ExternalInput")
    b1_dr = nc.dram_tensor("bn1_b", [64], F32, kind="ExternalInput")
    w2_dr = nc.dram_tensor("conv2_w", [128, 64, 3, 3], F32, kind="ExternalInput")
    g2_dr = nc.dram_tensor("bn2_g", [128], F32, kind="ExternalInput")
    b2_dr = nc.dram_tensor("bn2_b", [128], F32, kind="ExternalInput")
    fc1_dr = nc.dram_tensor("fc1_w", [256, 8192], F32, kind="ExternalInput")
    fc2_dr = nc.dram_tensor("fc2_w", [10, 256], F32, kind="ExternalInput")
    fc2b_dr = nc.dram_tensor("fc2_b", [10], F32, kind="ExternalInput")
    out_dr = nc.dram_tensor("out", [T, BL, 10], F32, kind="ExternalOutput")

    with tile.TileContext(nc) as tc:
        import contextlib
        with contextlib.ExitStack() as ctx:
            dram = ctx.enter_context(tc.tile_pool(name="dram", bufs=1, space="DRAM"))
            persist = ctx.enter_context(tc.tile_pool(name="persist", bufs=1))

            # internal DRAM for layer outputs (pre-BN conv results, fp32)
            y1_dram = dram.tile([8, 16, 128, 512], F32)   # [pair, seg=(t,hh), part, col]
            y2_dram = dram.tile([T, BL, 128, 256], F32)   # [t, b, ch, hw]

            # persistent small tensors
            scale1 = persist.tile([128, T], F32)
            bias1 = persist.tile([128, T], F32)
            scale2 = persist.tile([128, T], F32)
            bias2 = persist.tile([128, T], F32)
            s1buf = persist.tile([128, 128], F32)
            s2buf = persist.tile([128, 128], F32)
            s1buf2 = persist.tile([128, 64], F32)
            s2buf2 = persist.tile([128, 64], F32)

            # conv1 weights: row r = dx*9+dy*3+ci, duplicated on 4 strips
            w1f = persist.tile([27, 64], F32)
            for dy in range(3):
                for dx in range(3):
                    r0 = dy * 9 + dx * 3
                    nc.sync.dma_start(
                        w1f[r0 : r0 + 3, :],
                        w1_dr.ap()[:, :, dy, dx].rearrange("c ci -> ci c"),
                    )
            w1_hi = persist.tile([128, 64], BF16)
            w1_lo = persist.tile([128, 64], BF16)
            nc.vector.tensor_copy(w1_hi[0:27], w1f[:])
            nc.vector.tensor_tensor(w1_lo[0:27], w1f[:], w1_hi[0:27], ALU.subtract)
            for j in (32, 64, 96):
                nc.sync.dma_start(w1_hi[j : j + 27], w1_hi[0:27])
                nc.sync.dma_start(w1_lo[j : j + 27], w1_lo[0:27])

            # conv2 weights per shift-group g=(dy,dx): [64ci, 9g, 128c], dup parity
            w2f = persist.tile([64, 9, 128], F32)
            nc.sync.dma_start(w2f[:], w2_dr.ap().rearrange("c ci dy dx -> ci (dy dx) c"))
            w2_hi = persist.tile([128, 9, 128], BF16)
            w2_lo = persist.tile([128, 9, 128], BF16)
            nc.vector.tensor_copy(w2_hi[0:64], w2f[:])
            nc.vector.tensor_tensor(w2_lo[0:64], w2f[:], w2_hi[0:64], ALU.subtract)
            nc.sync.dma_start(w2_hi[64:128], w2_hi[0:64])
            nc.sync.dma_start(w2_lo[64:128], w2_lo[0:64])

            # =============== STAGE A: conv1 + stats + store ===============
            # Padded per-channel planes staged in DRAM; each im2col row is a
            # contiguous shifted window of a plane -> few large DMAs.
            t_order = (0, 2, 4, 6, 1, 3, 5, 7)  # rotate im2col strips
            GUARD = 64
            PLANE = 128 * 1156  # (t b) frames of 34x34
            # row r = dy*9 + dx*3 + ci holds plane ci pre-shifted by
            # (dy-1)*34 + (dx-1): im2col strips then load as single wide DMAs.
            xflat_hi = dram.tile([27, GUARD + PLANE + GUARD], BF16)
            xflat_lo = dram.tile([27, GUARD + PLANE + GUARD], BF16)
            with tc.tile_pool(name="psumA", bufs=8, space="PSUM") as psum, \
                 tc.tile_pool(name="ysb", bufs=4) as ysb_pool, \
                 tc.tile_pool(name="sq", bufs=2) as sq_pool, \
                 tc.tile_pool(name="imc", bufs=1) as imc_pool:
                with tc.tile_pool(name="xstage", bufs=1) as xst:
                    # x staged with (t b) on partitions: padded frames are
                    # contiguous per partition -> 2.3KB DMA runs to DRAM.
                    xpadF = xst.tile([128, 3, 34, 34], F32)
                    nc.vector.memset(xpadF[:], 0.0)
                    for ci in range(3):
                        nc.sync.dma_start(
                            xpadF[:, ci, 1:33, 1:33],
                            x_seq.ap()[:, :, ci].rearrange("t b h w -> (t b) h w"),
                        )
                    xpad_flat = xpadF.rearrange("p c h w -> p (c h w)")
                    x_hiF = xst.tile([128, 3, 34, 34], BF16)
                    x_loF = xst.tile([128, 3, 34, 34], BF16)
                    xhi_flat = x_hiF.rearrange("p c h w -> p (c h w)")
                    xlo_flat = x_loF.rearrange("p c h w -> p (c h w)")
                    nc.vector.tensor_copy(xhi_flat[:], xpad_flat[:])
                    nc.vector.tensor_tensor(xlo_flat[:], xpad_flat[:],
                                            xhi_flat[:], ALU.subtract)
                    _qs = (nc.sync, nc.scalar)
                    _qi = 0
                    for dy in range(3):
                        for dx in range(3):
                            for ci in range(3):
                                r = dy * 9 + dx * 3 + ci
                                shift = (dy - 1) * 34 + (dx - 1)
                                for src_sb, dst_dr in ((x_hiF, xflat_hi),
                                                       (x_loF, xflat_lo)):
                                    _qs[_qi % 2].dma_start(
                                        dst_dr[r, GUARD - shift :
                                               GUARD - shift + PLANE]
                                            .rearrange("(tb f) -> tb f", f=1156),
                                        src_sb[:, ci].rearrange("p h w -> p (h w)"),
                                    )
                                    _qi += 1

                # im2col strips: strip j (partitions 32j..32j+26) holds t in
                # {2j, 2j+1}; row r = dy*9 + dx*3 + ci; cols = padded frames.
                # Each (j,dy,dx) row-triple is one contiguous window per plane.
                SLEN = 2 * 16 * 1156  # 36992 cols per strip
                imc_hi = imc_pool.tile([128, SLEN], BF16)
                imc_lo = imc_pool.tile([128, SLEN], BF16)
                for j in range(4):
                    off = GUARD + 2 * j * 16 * 1156
                    nc.sync.dma_start(
                        imc_hi[32 * j : 32 * j + 27, :],
                        xflat_hi[:, off : off + SLEN],
                    )
                    nc.scalar.dma_start(
                        imc_lo[32 * j : 32 * j + 27, :],
                        xflat_lo[:, off : off + SLEN],
                    )

                ihi_v = imc_hi.rearrange("p (tb h w) -> p tb h w", h=34, w=34)
                ilo_v = imc_lo.rearrange("p (tb h w) -> p tb h w", h=34, w=34)
                for idx in range(128):
                    p = idx // 16
                    t = t_order[idx % 8]
                    hh = (idx // 8) % 2
                    j = t // 2
                    ps = psum.tile([128, 512], F32, tag="ps")
                    for half in range(2):
                        b = 2 * p + half
                        tbi = (t % 2) * 16 + b
                        h0 = hh * 16
                        args = [
                            (w1_hi, ihi_v), (w1_lo, ihi_v), (w1_hi, ilo_v),
                        ]
                        for k, (wt, im) in enumerate(args):
                            nc.tensor.matmul(
                                ps[64 * half : 64 * half + 64, :],
                                wt[32 * j : 32 * j + 27, :],
                                im[32 * j : 32 * j + 27, tbi,
                                   h0 + 1 : h0 + 17, 1:33],
                                start=(k == 0), stop=(k == 2),
                                tile_position=(32 * j, 64 * half),
                            )
                    y_sb = ysb_pool.tile([128, 512], F32)
                    nc.scalar.activation(y_sb[:], ps[:], AF.Identity,
                                         bias=0.0, scale=1.0,
                                         accum_out=s1buf[:, idx : idx + 1])
                    sq = sq_pool.tile([128, 512], F32)
                    nc.scalar.activation(sq[:], ps[:], AF.Square,
                                         bias=0.0, scale=1.0,
                                         accum_out=s2buf[:, idx : idx + 1])
                    seg = t * 2 + hh
                    (nc.sync if idx % 2 == 0 else nc.scalar).dma_start(
                        y1_dram[p, seg], y_sb[:])

            # =============== BN1 stats + allreduce ===============
            sums1 = persist.tile([128, 2], F32)
            nc.vector.tensor_reduce(sums1[:, 0:1], s1buf[:], mybir.AxisListType.X, ALU.add)
            nc.vector.tensor_reduce(sums1[:, 1:2], s2buf[:], mybir.AxisListType.X, ALU.add)
            g1 = _allreduce(nc, dram, persist, sums1[:], (128, 2))
            par1 = persist.tile([64, 2], F32)
            nc.sync.dma_start(par1[:], g1[64:128, :])
            tot1 = persist.tile([64, 2], F32)
            nc.vector.tensor_tensor(tot1[:], g1[0:64, :], par1[:], ALU.add)
            _stats_to_scale_bias(nc, persist, tot1, g1_dr, b1_dr,
                                 float(T * B_FULL * 32 * 32), 64, scale1, bias1)
            nc.sync.dma_start(scale1[64:128, :], scale1[0:64, :])
            nc.sync.dma_start(bias1[64:128, :], bias1[0:64, :])

            # fc weights + pooled2 (allocated after stage A frees its SBUF)
            fcpool = ctx.enter_context(tc.tile_pool(name="fcpool", bufs=1))
            fc1w = fcpool.tile([128, 256, 64], F32)  # [r, o, k] ; i = r*64+k
            nc.sync.dma_start(
                fc1w[:], fc1_dr.ap().rearrange("o (r k) -> r o k", r=128)
            )
            fc2w = fcpool.tile([128, 2, 10], F32)  # [r, m, o] ; i = m*128+r
            for m in range(2):
                nc.sync.dma_start(
                    fc2w[:, m, :],
                    fc2_dr.ap()[:, m * 128 : (m + 1) * 128].rearrange("o r -> r o"),
                )
            fc2b = fcpool.tile([10, 1], F32)
            nc.sync.dma_start(fc2b[:], fc2b_dr.ap()[:, None])
            pooled2 = fcpool.tile([128, 8192], F32)  # [(c), (t b hw)]

            # =============== STAGE B: LIF1 + pool ===============
            with tc.tile_pool(name="pooled1_pool", bufs=1) as pp1:
                pooled1 = pp1.tile([128, T, 8, 18, 18], BF16)
                nc.vector.memset(pooled1[:], 0.0)
                with tc.tile_pool(name="stageB", bufs=3) as pB, \
                     tc.tile_pool(name="stateB", bufs=1) as stB:
                    for p in range(8):
                        st = [stB.tile([128, 1024], F32, tag=f"st{i}", name=f"stB{i}") for i in range(2)]
                        nc.vector.memset(st[0][:], 0.0)
                        for t in range(T):
                            yc = pB.tile([128, 2, 512], F32, tag="yc", name="ycB")
                            nc.sync.dma_start(
                                yc[:],
                                y1_dram[p, 2 * t : 2 * t + 2].rearrange(
                                    "s part c -> part s c"),
                            )
                            a_new, a_old = st[(t + 1) % 2], st[t % 2]
                            nc.vector._custom_dve(
                                LIF_CHARGE, out=a_new[:], in0=a_old[:],
                                in1=yc.rearrange("p s c -> p (s c)"),
                                s0=scale1[:, t : t + 1], s1=bias1[:, t : t + 1],
                                imm2=float(2.0**t),
                            )
                            av = a_new.rearrange("p (h w) -> p h w", h=32)
                            hp = pB.tile([128, 32, 16], F32, tag="hp")
                            nc.vector._custom_dve(
                                SPIKE_HPOOL, out=hp[:],
                                in0=av[:, :, 0:32:2], in1=av[:, :, 1:32:2],
                                s0=float(2.0 ** (t + 1)),
                            )
                            nc.vector._custom_dve(
                                VPOOL_SCALE,
                                out=pooled1[:, t, p, 1:17, 1:17],
                                in0=hp[:, 0:32:2, :], in1=hp[:, 1:32:2, :],
                                s1=0.25,
                            )

                # =============== STAGE C: conv2 + stats + store ===============
                with tc.tile_pool(name="ysb2", bufs=4) as ysb2_pool, \
                     tc.tile_pool(name="psumC", bufs=8, space="PSUM") as psum, \
                     tc.tile_pool(name="sq2", bufs=2) as sq2_pool:
                    cidx = 0
                    for t in range(T):
                        for p0 in (0, 2, 4, 6):
                            for par in range(2):
                                ps = psum.tile([128, 512], F32, tag="ps")
                                k = 0
                                for wt in (w2_hi, w2_lo):
                                    for g in range(9):
                                        dy, dx = g // 3, g % 3
                                        rhs = pooled1[64 * par : 64 * par + 64, t,
                                                      p0 : p0 + 2,
                                                      dy : dy + 16, dx : dx + 16]
                                        nc.tensor.matmul(
                                            ps[:],
                                            wt[64 * par : 64 * par + 64, g, :],
                                            rhs,
                                            start=(k == 0), stop=(k == 17),
                                        )
                                        k += 1
                                y_sb = ysb2_pool.tile([128, 512], F32)
                                nc.scalar.activation(y_sb[:], ps[:], AF.Identity,
                                                     bias=0.0, scale=1.0,
                                                     accum_out=s1buf2[:, cidx : cidx + 1])
                                sq = sq2_pool.tile([128, 512], F32)
                                nc.scalar.activation(sq[:], ps[:], AF.Square,
                                                     bias=0.0, scale=1.0,
                                                     accum_out=s2buf2[:, cidx : cidx + 1])
                                cidx += 1
                                b0 = 2 * p0 + par
                                nc.sync.dma_start(
                                    y2_dram[t, b0 : b0 + 3 : 2].rearrange(
                                        "b p c -> p b c"),
                                    y_sb.rearrange("p (b c) -> p b c", b=2),
                                )

            # =============== BN2 stats + allreduce ===============
            sums2 = persist.tile([128, 2], F32)
            nc.vector.tensor_reduce(sums2[:, 0:1], s1buf2[:], mybir.AxisListType.X, ALU.add)
            nc.vector.tensor_reduce(sums2[:, 1:2], s2buf2[:], mybir.AxisListType.X, ALU.add)
            g2 = _allreduce(nc, dram, persist, sums2[:], (128, 2))
            _stats_to_scale_bias(nc, persist, g2, g2_dr, b2_dr,
                                 float(T * B_FULL * 16 * 16), 128, scale2, bias2)

            # =============== STAGE D: LIF2 + pool ===============
            with tc.tile_pool(name="stageD", bufs=2) as pD, \
                 tc.tile_pool(name="stateD", bufs=1) as stD:
                for bp in range(8):  # b-pairs
                    b0 = 2 * bp
                    yc = pD.tile([128, T, 2, 256], F32)
                    for t in range(T):
                        nc.sync.dma_start(
                            yc[:, t],
                            y2_dram[t, b0 : b0 + 2].rearrange("b p c -> p b c"),
                        )
                    ycv = yc.rearrange("p t b c -> p t (b c)")
                    st = [stD.tile([128, 512], F32, tag=f"std{i}", name=f"stD{i}") for i in range(2)]
                    nc.vector.memset(st[0][:], 0.0)
                    for t in range(T):
                        a_new, a_old = st[(t + 1) % 2], st[t % 2]
                        nc.vector._custom_dve(
                            LIF_CHARGE, out=a_new[:], in0=a_old[:],
                            in1=ycv[:, t, :],
                            s0=scale2[:, t : t + 1], s1=bias2[:, t : t + 1],
                            imm2=float(2.0**t),
                        )
                        av = a_new.rearrange("p (bh w) -> p bh w", w=16)
                        hp = pD.tile([128, 32, 8], F32, tag="hp2")
                        nc.vector._custom_dve(
                            SPIKE_HPOOL, out=hp[:],
                            in0=av[:, :, 0:16:2], in1=av[:, :, 1:16:2],
                            s0=float(2.0 ** (t + 1)),
                        )
                        pout = pooled2[:, (t * 16 + b0) * 64 : (t * 16 + b0 + 2) * 64]
                        nc.vector._custom_dve(
                            VPOOL_SCALE,
                            out=pout.rearrange("p (bh w) -> p bh w", w=8),
                            in0=hp[:, 0:32:2, :], in1=hp[:, 1:32:2, :],
                            s1=0.25,
                        )

            # =============== STAGE E: fc1 + LIF + fc2 ===============
            p2v = pooled2.rearrange("p (tb k) -> p tb k", k=64)
            with tc.tile_pool(name="stageE", bufs=1) as pE, \
                 tc.tile_pool(name="psumE", bufs=2, space="PSUM") as psE:
                s_sb = pE.tile([128, 2, T, BL], F32)
                for m in range(2):
                    psf = psE.tile([128, 128], F32, tag="psf")
                    for k in range(64):
                        nc.tensor.matmul(
                            psf[:], fc1w[:, m * 128 : (m + 1) * 128, k],
                            p2v[:, :, k],
                            start=(k == 0), stop=(k == 63),
                        )
                    stf = [pE.tile([128, BL], F32, tag=f"stf{i}", name=f"stf{i}") for i in range(2)]
                    nc.vector.memset(stf[0][:], 0.0)
                    for t in range(T):
                        a_new, a_old = stf[(t + 1) % 2], stf[t % 2]
                        nc.vector._custom_dve(
                            LIF_CHARGE, out=a_new[:], in0=a_old[:],
                            in1=psf[:, t * BL : (t + 1) * BL],
                            s0=float(2.0**t), s1=0.0, imm2=float(2.0**t),
                        )
                        nc.vector._custom_dve(
                            SPIKE_GE, out=s_sb[:, m, t, :], in0=a_new[:],
                            s0=float(2.0 ** (t + 1)),
                        )
                pso = psE.tile([10, 128], F32, tag="pso")
                sv = s_sb.rearrange("p m t b -> p m (t b)")
                nc.tensor.matmul(pso[:], fc2w[:, 0, :], sv[:, 0, :],
                                 start=True, stop=False)
                nc.tensor.matmul(pso[:], fc2w[:, 1, :], sv[:, 1, :],
                                 start=False, stop=True)
                out_sb = pE.tile([10, 128], F32)
                nc.scalar.activation(out_sb[:], pso[:], AF.Identity,
                                     bias=fc2b[:, 0:1], scale=1.0)
                nc.sync.dma_start(out_dr.ap().rearrange("t b o -> o (t b)"), out_sb[:])

    return nc


_CACHED = None


def _get_compiled():
    global _CACHED
    if _CACHED is None:
        nc = bacc.Bacc("TRN2", target_bir_lowering=False, debug=False,
                       num_devices=N_CORES)
        build(nc)
        nc.compile()
        _CACHED = nc
    return _CACHED


def kernel(**inputs) -> np.ndarray:
    nc = _get_compiled()
    np_in = {k: np.ascontiguousarray(np.asarray(v, dtype=np.float32))
             for k, v in inputs.items()}
    in_maps = []
    for i in range(N_CORES):
        m = dict(np_in)
        m["x_seq"] = np.ascontiguousarray(
            np_in["x_seq"][:, i * BL : (i + 1) * BL])
        in_maps.append(m)
    res = bass_utils.run_bass_kernel_spmd(nc, in_maps, core_ids=list(range(N_CORES)))
    return np.concatenate([res.results[i]["out"] for i in range(N_CORES)], axis=1)


if __name__ == "__main__":
    nc = _get_compiled()
    print("compiled OK")


# revision 17
# speedup vs baseline: 1.4002x; 1.1139x over previous
"""Trainium2 Bass kernel for nn_CifarBaselineSNN.

conv1(3->64,3x3,p1) -> BN -> LIF -> avgpool2
conv2(64->128,3x3,p1) -> BN -> LIF -> avgpool2
fc1(8192->256) -> LIF -> fc2(256->10)+b
T=8, B=128. Data-parallel over B across 8 NeuronCores (16 samples/core);
BN statistics are global over the batch -> 2 small AllReduces.

Numerics: convolutions run as bf16 hi/lo weight-split matmuls accumulating in
fp32 PSUM (inputs to conv2 are pooled spikes, exactly representable in bf16;
conv1 inputs are hi/lo split too). LIF state uses the 2^t scaling trick so one
fused DVE op does decay+charge+reset per step.
"""

import sys
import os

for _p in ("/opt/trn_rl_repo", "/root/.axon_site/_ro/trn_rl_repo"):
    if os.path.isdir(_p) and _p not in sys.path:
        sys.path.append(_p)

import numpy as np

import concourse.bass as bass
import concourse.mybir as mybir
import concourse.tile as tile
from concourse import bacc
from concourse import bass_utils
from concourse import dve_ops as _dops
from concourse.dve_uop import DveOpSpec
from concourse.dve_spec import (
    Spec, Src0, Src1, C0, C1, C2, Zero, AluOp, sq, select, lower, _has_src1,
)

F32 = mybir.dt.float32
BF16 = mybir.dt.bfloat16
AF = mybir.ActivationFunctionType
ALU = mybir.AluOpType

T = 8
B_FULL = 128
N_CORES = 8
BL = B_FULL // N_CORES  # 16 samples per core
EPS = 1e-5


# --------------------------------------------------------------------------
# Custom DVE ops (fused LIF pieces)
# --------------------------------------------------------------------------

def _register_op(name, spec, ref):
    for op in _dops.OPS:
        if op.name == name:
            return op
    spec = Spec(body=spec.body, accum=spec.accum, accum_init=spec.accum_init,
                reference=ref)
    shas = {}
    for ver in ("v3", "v4"):
        s = DveOpSpec(name=name, opcode=0, uops=lower(spec, ver=ver),
                      rd1_en=_has_src1(spec))
        shas[ver] = s.sha(ver)
    op = _dops.DveOp(name, spec, subdim=False, uops_sha=shas)
    _dops.OPS.append(op)
    _dops.CUSTOM_DVE_SPECS[name] = spec
    _dops._SUB_OPCODE_FOR_NAME[name] = max(_dops._SUB_OPCODE_FOR_NAME.values()) + 1
    return op


# A_t = (A_{t-1} if A_{t-1} < theta_{t-1} else 0) + y*scale + bias
LIF_CHARGE = _register_op(
    "LIF_CHARGE_SNN",
    Spec(body=select(Src0 >= C2, Zero, Src0) + Src1 * C0 + C1),
    lambda in0, in1, s0, s1, imm2: np.where(in0 >= imm2, 0.0, in0) + in1 * s0 + s1,
)

# spike counts over horizontal pairs: (a>=th) + (b>=th)   (values 0/1/2)
SPIKE_HPOOL = _register_op(
    "SPIKE_HPOOL_SNN",
    Spec(body=(Src0 >= C0) + (Src1 >= C0)),
    lambda in0, in1, s0, s1, imm2: (in0 >= s0).astype(np.float32)
    + (in1 >= s0).astype(np.float32),
)

# pooled = (hpA + hpB) * 0.25
VPOOL_SCALE = _register_op(
    "VPOOL_SCALE_SNN",
    Spec(body=(Src0 + Src1) * C1),
    lambda in0, in1, s0, s1, imm2: (in0 + in1) * s1,
)

# square + row-sum (for BN sum-of-squares on the vector engine)
SQ_ACC = _register_op(
    "SQ_ACC_SNN",
    Spec(body=sq(Src0), accum=AluOp.ADD),
    lambda in0, s0, s1, imm2: in0 * in0,
)

# plain spike: (a >= th)
SPIKE_GE = _register_op(
    "SPIKE_GE_SNN",
    Spec(body=(Src0 >= C0) + Zero),
    lambda in0, s0, s1, imm2: (in0 >= s0).astype(np.float32),
)


# --------------------------------------------------------------------------
# Kernel build
# --------------------------------------------------------------------------

def _stats_to_scale_bias(nc, pool, tot, g_dr, b_dr, n_count, nch, out_scale, out_bias):
    """tot: [nch,2] SBUF (sum, sumsq). Writes out_scale/out_bias [nch,8]:
    scale[:,t] = gamma*rstd*2^t ; bias[:,t] = (beta - mu*gamma*rstd)*2^t."""
    mu = pool.tile([nch, 1], F32)
    nc.vector.tensor_scalar_mul(mu[:], tot[:, 0:1], 1.0 / n_count)
    e2 = pool.tile([nch, 1], F32)
    nc.vector.tensor_scalar_mul(e2[:], tot[:, 1:2], 1.0 / n_count)
    var = pool.tile([nch, 1], F32)
    nc.vector.tensor_tensor(var[:], mu[:], mu[:], ALU.mult)
    nc.vector.tensor_tensor(var[:], e2[:], var[:], ALU.subtract)
    nc.vector.tensor_scalar_add(var[:], var[:], float(EPS))
    std = pool.tile([nch, 1], F32)
    nc.scalar.activation(std[:], var[:], AF.Sqrt, bias=0.0, scale=1.0)
    rstd = pool.tile([nch, 1], F32)
    nc.vector.reciprocal(rstd[:], std[:])
    gam = pool.tile([nch, 1], F32)
    nc.sync.dma_start(gam[:], g_dr.ap()[:, None])
    bet = pool.tile([nch, 1], F32)
    nc.sync.dma_start(bet[:], b_dr.ap()[:, None])
    gr = pool.tile([nch, 1], F32)
    nc.vector.tensor_tensor(gr[:], gam[:], rstd[:], ALU.mult)
    bb = pool.tile([nch, 1], F32)  # beta - mu*gr
    nc.vector.tensor_tensor(bb[:], mu[:], gr[:], ALU.mult)
    nc.vector.tensor_tensor(bb[:], bet[:], bb[:], ALU.subtract)
    for t in range(T):
        nc.vector.tensor_scalar_mul(out_scale[:nch, t : t + 1], gr[:], float(2.0**t))
        nc.vector.tensor_scalar_mul(out_bias[:nch, t : t + 1], bb[:], float(2.0**t))


def _allreduce(nc, dram_pool, sb_pool, src_ap, shape):
    """AllReduce-add src_ap ([P,F] SBUF) across all 8 cores; returns SBUF tile."""
    bin_ = dram_pool.tile(list(shape), F32)
    bout = dram_pool.tile(list(shape), F32)
    nc.gpsimd.dma_start(bin_[:], src_ap)
    nc.gpsimd.collective_compute(
        "AllReduce", ALU.add,
        replica_groups=[list(range(N_CORES))],
        ins=[bin_.opt()], outs=[bout.opt()],
    )
    res = sb_pool.tile(list(shape), F32)
    nc.gpsimd.dma_start(res[:], bout[:])
    return res


def build(nc):
    # ---- DRAM I/O -------------------------------------------------------
    x_seq = nc.dram_tensor("x_seq", [T, BL, 3, 32, 32], F32, kind="ExternalInput")
    w1_dr = nc.dram_tensor("conv1_w", [64, 3, 3, 3], F32, kind="ExternalInput")
    g1_dr = nc.dram_tensor("bn1_g", [64], F32, kind="ExternalInput")
    b1_dr = nc.dram_tensor("bn1_b", [64], F32, kind="ExternalInput")
    w2_dr = nc.dram_tensor("conv2_w", [128, 64, 3, 3], F32, kind="ExternalInput")
    g2_dr = nc.dram_tensor("bn2_g", [128], F32, kind="ExternalInput")
    b2_dr = nc.dram_tensor("bn2_b", [128], F32, kind="ExternalInput")
    fc1_dr = nc.dram_tensor("fc1_w", [256, 8192], F32, kind="ExternalInput")
    fc2_dr = nc.dram_tensor("fc2_w", [10, 256], F32, kind="ExternalInput")
    fc2b_dr = nc.dram_tensor("fc2_b", [10], F32, kind="ExternalInput")
    out_dr = nc.dram_tensor("out", [T, BL, 10], F32, kind="ExternalOutput")

    with tile.TileContext(nc) as tc:
        import contextlib
        with contextlib.ExitStack() as ctx:
            dram = ctx.enter_context(tc.tile_pool(name="dram", bufs=1, space="DRAM"))
            persist = ctx.enter_context(tc.tile_pool(name="persist", bufs=1))

            # internal DRAM for layer outputs (pre-BN conv results, fp32)
            y1_dram = dram.tile([8, 16, 128, 512], F32)   # [pair, seg=(t,hh), part, col]
            y2_dram = dram.tile([T, BL, 128, 256], F32)   # [t, b, ch, hw]

            # persistent small tensors
            scale1 = persist.tile([128, T], F32)
            bias1 = persist.tile([128, T], F32)
            scale2 = persist.tile([128, T], F32)
            bias2 = persist.tile([128, T], F32)
            s1buf = persist.tile([128, 128], F32)
            s2buf = persist.tile([128, 128], F32)
            s1buf2 = persist.tile([128, 64], F32)
            s2buf2 = persist.tile([128, 64], F32)

            # conv1 weights: row r = dx*9+dy*3+ci, duplicated on 4 strips
            w1f = persist.tile([27, 64], F32)
            for dy in range(3):
                for dx in range(3):
                    r0 = dy * 9 + dx * 3
                    nc.sync.dma_start(
                        w1f[r0 : r0 + 3, :],
                        w1_dr.ap()[:, :, dy, dx].rearrange("c ci -> ci c"),
                    )
            w1_hi = persist.tile([128, 64], BF16)
            w1_lo = persist.tile([128, 64], BF16)
            nc.vector.tensor_copy(w1_hi[0:27], w1f[:])
            nc.vector.tensor_tensor(w1_lo[0:27], w1f[:], w1_hi[0:27], ALU.subtract)
            for j in (32, 64, 96):
                nc.sync.dma_start(w1_hi[j : j + 27], w1_hi[0:27])
                nc.sync.dma_start(w1_lo[j : j + 27], w1_lo[0:27])

            # conv2 weights per shift-group g=(dy,dx): [64ci, 9g, 128c], dup parity
            w2f = persist.tile([64, 9, 128], F32)
            nc.sync.dma_start(w2f[:], w2_dr.ap().rearrange("c ci dy dx -> ci (dy dx) c"))
            w2_hi = persist.tile([128, 9, 128], BF16)
            w2_lo = persist.tile([128, 9, 128], BF16)
            nc.vector.tensor_copy(w2_hi[0:64], w2f[:])
            nc.vector.tensor_tensor(w2_lo[0:64], w2f[:], w2_hi[0:64], ALU.subtract)
            nc.sync.dma_start(w2_hi[64:128], w2_hi[0:64])
            nc.sync.dma_start(w2_lo[64:128], w2_lo[0:64])

            # =============== STAGE A: conv1 + stats + store ===============
            # Padded per-channel planes staged in DRAM; each im2col row is a
            # contiguous shifted window of a plane -> few large DMAs.
            t_order = (0, 2, 4, 6, 1, 3, 5, 7)  # rotate im2col strips
            GUARD = 64
            PLANE = 128 * 1156  # (t b) frames of 34x34
            # row r = dy*9 + dx*3 + ci holds plane ci pre-shifted by
            # (dy-1)*34 + (dx-1): im2col strips then load as single wide DMAs.
            xflat_hi = dram.tile([27, GUARD + PLANE + GUARD], BF16)
            xflat_lo = dram.tile([27, GUARD + PLANE + GUARD], BF16)
            with tc.tile_pool(name="psumA", bufs=8, space="PSUM") as psum, \
                 tc.tile_pool(name="ysb", bufs=4) as ysb_pool, \
                 tc.tile_pool(name="sq", bufs=2) as sq_pool, \
                 tc.tile_pool(name="imc", bufs=1) as imc_pool:
                with tc.tile_pool(name="xstage", bufs=1) as xst:
                    # x staged with (t b) on partitions: padded frames are
                    # contiguous per partition -> 2.3KB DMA runs to DRAM.
                    xpadF = xst.tile([128, 3, 34, 34], F32)
                    nc.vector.memset(xpadF[:], 0.0)
                    for ci in range(3):
                        nc.sync.dma_start(
                            xpadF[:, ci, 1:33, 1:33],
                            x_seq.ap()[:, :, ci].rearrange("t b h w -> (t b) h w"),
                        )
                    xpad_flat = xpadF.rearrange("p c h w -> p (c h w)")
                    x_hiF = xst.tile([128, 3, 34, 34], BF16)
                    x_loF = xst.tile([128, 3, 34, 34], BF16)
                    xhi_flat = x_hiF.rearrange("p c h w -> p (c h w)")
                    xlo_flat = x_loF.rearrange("p c h w -> p (c h w)")
                    nc.vector.tensor_copy(xhi_flat[:], xpad_flat[:])
                    nc.vector.tensor_tensor(xlo_flat[:], xpad_flat[:],
                                            xhi_flat[:], ALU.subtract)
                    _qs = (nc.sync, nc.scalar)
                    _qi = 0
                    for dy in range(3):
                        for dx in range(3):
                            for ci in range(3):
                                r = dy * 9 + dx * 3 + ci
                                shift = (dy - 1) * 34 + (dx - 1)
                                for src_sb, dst_dr in ((x_hiF, xflat_hi),
                                                       (x_loF, xflat_lo)):
                                    _qs[_qi % 2].dma_start(
                                        dst_dr[r, GUARD - shift :
                                               GUARD - shift + PLANE]
                                            .rearrange("(tb f) -> tb f", f=1156),
                                        src_sb[:, ci].rearrange("p h w -> p (h w)"),
                                    )
                                    _qi += 1

                # im2col strips: strip j (partitions 32j..32j+26) holds t in
                # {2j, 2j+1}; row r = dy*9 + dx*3 + ci; cols = padded frames.
                # Each (j,dy,dx) row-triple is one contiguous window per plane.
                SLEN = 2 * 16 * 1156  # 36992 cols per strip
                imc_hi = imc_pool.tile([128, SLEN], BF16)
                imc_lo = imc_pool.tile([128, SLEN], BF16)
                for j in range(4):
                    off = GUARD + 2 * j * 16 * 1156
                    nc.sync.dma_start(
                        imc_hi[32 * j : 32 * j + 27, :],
                        xflat_hi[:, off : off + SLEN],
                    )
                    nc.scalar.dma_start(
                        imc_lo[32 * j : 32 * j + 27, :],
                        xflat_lo[:, off : off + SLEN],
                    )

                ihi_v = imc_hi.rearrange("p (tb h w) -> p tb h w", h=34, w=34)
                ilo_v = imc_lo.rearrange("p (tb h w) -> p tb h w", h=34, w=34)
                for grp in range(32):
                    p = grp // 4
                    hh = (grp // 2) % 2
                    phase = grp % 2
                    ts4 = (0, 2, 4, 6) if phase == 0 else (1, 3, 5, 7)
                    pss = [psum.tile([128, 512], F32, tag="ps", name=f"ps{i}")
                           for i in range(4)]
                    h0 = hh * 16
                    args = [(w1_hi, ihi_v), (w1_lo, ihi_v), (w1_hi, ilo_v)]
                    for k, (wt, im) in enumerate(args):
                        for half in range(2):
                            b = 2 * p + half
                            for i, t in enumerate(ts4):
                                j = t // 2
                                tbi = (t % 2) * 16 + b
                                nc.tensor.matmul(
                                    pss[i][64 * half : 64 * half + 64, :],
                                    wt[32 * j : 32 * j + 27, :],
                                    im[32 * j : 32 * j + 27, tbi,
                                       h0 + 1 : h0 + 17, 1:33],
                                    start=(k == 0), stop=(k == 2),
                                    tile_position=(32 * j, 64 * half),
                                )
                    for i, t in enumerate(ts4):
                        idx = grp * 4 + i
                        y_sb = ysb_pool.tile([128, 512], F32)
                        nc.scalar.activation(y_sb[:], pss[i][:], AF.Identity,
                                             bias=0.0, scale=1.0,
                                             accum_out=s1buf[:, idx : idx + 1])
                        sq_t = sq_pool.tile([128, 512], F32, name="sqsc")
                        nc.vector._custom_dve(
                            SQ_ACC, out=sq_t[:], in0=pss[i][:],
                            accum_out=s2buf[:, idx : idx + 1])
                        seg = t * 2 + hh
                        (nc.sync if idx % 2 == 0 else nc.scalar).dma_start(
                            y1_dram[p, seg], y_sb[:])

            # =============== BN1 stats + allreduce ===============
            sums1 = persist.tile([128, 2], F32)
            nc.vector.tensor_reduce(sums1[:, 0:1], s1buf[:], mybir.AxisListType.X, ALU.add)
            nc.vector.tensor_reduce(sums1[:, 1:2], s2buf[:], mybir.AxisListType.X, ALU.add)
            g1 = _allreduce(nc, dram, persist, sums1[:], (128, 2))
            par1 = persist.tile([64, 2], F32)
            nc.sync.dma_start(par1[:], g1[64:128, :])
            tot1 = persist.tile([64, 2], F32)
            nc.vector.tensor_tensor(tot1[:], g1[0:64, :], par1[:], ALU.add)
            _stats_to_scale_bias(nc, persist, tot1, g1_dr, b1_dr,
                                 float(T * B_FULL * 32 * 32), 64, scale1, bias1)
            nc.sync.dma_start(scale1[64:128, :], scale1[0:64, :])
            nc.sync.dma_start(bias1[64:128, :], bias1[0:64, :])

            # fc weights + pooled2 (allocated after stage A frees its SBUF)
            fcpool = ctx.enter_context(tc.tile_pool(name="fcpool", bufs=1))
            fc1w = fcpool.tile([128, 256, 64], F32)  # [r, o, k] ; i = r*64+k
            nc.sync.dma_start(
                fc1w[:], fc1_dr.ap().rearrange("o (r k) -> r o k", r=128)
            )
            fc2w = fcpool.tile([128, 2, 10], F32)  # [r, m, o] ; i = m*128+r
            for m in range(2):
                nc.sync.dma_start(
                    fc2w[:, m, :],
                    fc2_dr.ap()[:, m * 128 : (m + 1) * 128].rearrange("o r -> r o"),
                )
            fc2b = fcpool.tile([10, 1], F32)
            nc.sync.dma_start(fc2b[:], fc2b_dr.ap()[:, None])
            pooled2 = fcpool.tile([128, 8192], F32)  # [(c), (t b hw)]

            # =============== STAGE B: LIF1 + pool ===============
            with tc.tile_pool(name="pooled1_pool", bufs=1) as pp1:
                pooled1 = pp1.tile([128, T, 8, 18, 18], BF16)
                nc.vector.memset(pooled1[:], 0.0)
                with tc.tile_pool(name="stageB", bufs=3) as pB, \
                     tc.tile_pool(name="stateB", bufs=1) as stB:
                    for p in range(8):
                        st = [stB.tile([128, 1024], F32, tag=f"st{i}", name=f"stB{i}") for i in range(2)]
                        nc.vector.memset(st[0][:], 0.0)
                        for t in range(T):
                            yc = pB.tile([128, 2, 512], F32, tag="yc", name="ycB")
                            nc.sync.dma_start(
                                yc[:],
                                y1_dram[p, 2 * t : 2 * t + 2].rearrange(
                                    "s part c -> part s c"),
                            )
                            a_new, a_old = st[(t + 1) % 2], st[t % 2]
                            nc.vector._custom_dve(
                                LIF_CHARGE, out=a_new[:], in0=a_old[:],
                                in1=yc.rearrange("p s c -> p (s c)"),
                                s0=scale1[:, t : t + 1], s1=bias1[:, t : t + 1],
                                imm2=float(2.0**t),
                            )
                            av = a_new.rearrange("p (h w) -> p h w", h=32)
                            hp = pB.tile([128, 32, 16], F32, tag="hp")
                            nc.vector._custom_dve(
                                SPIKE_HPOOL, out=hp[:],
                                in0=av[:, :, 0:32:2], in1=av[:, :, 1:32:2],
                                s0=float(2.0 ** (t + 1)),
                            )
                            nc.vector._custom_dve(
                                VPOOL_SCALE,
                                out=pooled1[:, t, p, 1:17, 1:17],
                                in0=hp[:, 0:32:2, :], in1=hp[:, 1:32:2, :],
                                s1=0.25,
                            )

                # =============== STAGE C: conv2 + stats + store ===============
                with tc.tile_pool(name="ysb2", bufs=4) as ysb2_pool, \
                     tc.tile_pool(name="psumC", bufs=8, space="PSUM") as psum, \
                     tc.tile_pool(name="sq2", bufs=2) as sq2_pool:
                    cidx = 0
                    for t in range(T):
                        for p0 in (0, 2, 4, 6):
                            pstiles = [psum.tile([128, 512], F32, tag="ps",
                                                 name=f"psc{par}")
                                       for par in range(2)]
                            k = 0
                            for wt in (w2_hi, w2_lo):
                                for g in range(9):
                                    dy, dx = g // 3, g % 3
                                    for par in range(2):
                                        rhs = pooled1[64 * par : 64 * par + 64, t,
                                                      p0 : p0 + 2,
                                                      dy : dy + 16, dx : dx + 16]
                                        nc.tensor.matmul(
                                            pstiles[par][:],
                                            wt[64 * par : 64 * par + 64, g, :],
                                            rhs,
                                            start=(k == 0), stop=(k == 17),
                                        )
                                    k += 1
                            for par in range(2):
                                ps = pstiles[par]
                                y_sb = ysb2_pool.tile([128, 512], F32)
                                nc.scalar.activation(y_sb[:], ps[:], AF.Identity,
                                                     bias=0.0, scale=1.0,
                                                     accum_out=s1buf2[:, cidx : cidx + 1])
                                sq_t = sq2_pool.tile([128, 512], F32, name="sqsc2")
                                nc.scalar.activation(sq_t[:], ps[:], AF.Square,
                                                     bias=0.0, scale=1.0,
                                                     accum_out=s2buf2[:, cidx : cidx + 1])
                                b0 = 2 * p0 + par
                                (nc.sync if cidx % 2 == 0 else nc.scalar).dma_start(
                                    y2_dram[t, b0 : b0 + 3 : 2].rearrange(
                                        "b p c -> p b c"),
                                    y_sb.rearrange("p (b c) -> p b c", b=2),
                                )
                                cidx += 1

            # =============== BN2 stats + allreduce ===============
            sums2 = persist.tile([128, 2], F32)
            nc.vector.tensor_reduce(sums2[:, 0:1], s1buf2[:], mybir.AxisListType.X, ALU.add)
            nc.vector.tensor_reduce(sums2[:, 1:2], s2buf2[:], mybir.AxisListType.X, ALU.add)
            g2 = _allreduce(nc, dram, persist, sums2[:], (128, 2))
            _stats_to_scale_bias(nc, persist, g2, g2_dr, b2_dr,
                                 float(T * B_FULL * 16 * 16), 128, scale2, bias2)

            # =============== STAGE D: LIF2 + pool ===============
            with tc.tile_pool(name="stageD", bufs=2) as pD, \
                 tc.tile_pool(name="stateD", bufs=1) as stD:
                for bp in range(8):  # b-pairs
                    b0 = 2 * bp
                    yc = pD.tile([128, T, 2, 256], F32)
                    for t in range(T):
                        nc.sync.dma_start(
                            yc[:, t],
                            y2_dram[t, b0 : b0 + 2].rearrange("b p c -> p b c"),
                        )
                    ycv = yc.rearrange("p t b c -> p t (b c)")
                    st = [stD.tile([128, 512], F32, tag=f"std{i}", name=f"stD{i}") for i in range(2)]
                    nc.vector.memset(st[0][:], 0.0)
                    for t in range(T):
                        a_new, a_old = st[(t + 1) % 2], st[t % 2]
                        nc.vector._custom_dve(
                            LIF_CHARGE, out=a_new[:], in0=a_old[:],
                            in1=ycv[:, t, :],
                            s0=scale2[:, t : t + 1], s1=bias2[:, t : t + 1],
                            imm2=float(2.0**t),
                        )
                        av = a_new.rearrange("p (bh w) -> p bh w", w=16)
                        hp = pD.tile([128, 32, 8], F32, tag="hp2")
                        nc.vector._custom_dve(
                            SPIKE_HPOOL, out=hp[:],
                            in0=av[:, :, 0:16:2], in1=av[:, :, 1:16:2],
                            s0=float(2.0 ** (t + 1)),
                        )
                        pout = pooled2[:, (t * 16 + b0) * 64 : (t * 16 + b0 + 2) * 64]
                        nc.vector._custom_dve(
                            VPOOL_SCALE,
                            out=pout.rearrange("p (bh w) -> p bh w", w=8),
                            in0=hp[:, 0:32:2, :], in1=hp[:, 1:32:2, :],
                            s1=0.25,
                        )

            # =============== STAGE E: fc1 + LIF + fc2 ===============
            p2v = pooled2.rearrange("p (tb k) -> p tb k", k=64)
            with tc.tile_pool(name="stageE", bufs=1) as pE, \
                 tc.tile_pool(name="psumE", bufs=2, space="PSUM") as psE:
                s_sb = pE.tile([128, 2, T, BL], F32)
                for m in range(2):
                    psf = psE.tile([128, 128], F32, tag="psf")
                    for k in range(64):
                        nc.tensor.matmul(
                            psf[:], fc1w[:, m * 128 : (m + 1) * 128, k],
                            p2v[:, :, k],
                            start=(k == 0), stop=(k == 63),
                        )
                    stf = [pE.tile([128, BL], F32, tag=f"stf{i}", name=f"stf{i}") for i in range(2)]
                    nc.vector.memset(stf[0][:], 0.0)
                    for t in range(T):
                        a_new, a_old = stf[(t + 1) % 2], stf[t % 2]
                        nc.vector._custom_dve(
                            LIF_CHARGE, out=a_new[:], in0=a_old[:],
                            in1=psf[:, t * BL : (t + 1) * BL],
                            s0=float(2.0**t), s1=0.0, imm2=float(2.0**t),
                        )
                        nc.vector._custom_dve(
                            SPIKE_GE, out=s_sb[:, m, t, :], in0=a_new[:],
                            s0=float(2.0 ** (t + 1)),
                        )
                pso = psE.tile([10, 128], F32, tag="pso")
                sv = s_sb.rearrange("p m t b -> p m (t b)")
                nc.tensor.matmul(pso[:], fc2w[:, 0, :], sv[:, 0, :],
                                 start=True, stop=False)
                nc.tensor.matmul(pso[:], fc2w[:, 1, :], sv[:, 1, :],
                                 start=False, stop=True)
                out_sb = pE.tile([10, 128], F32)
                nc.scalar.activation(out_sb[:], pso[:], AF.Identity,
                                     bias=fc2b[:, 0:1], scale=1.0)
                nc.sync.dma_start(out_dr.ap().rearrange("t b o -> o (t b)"), out_sb[:])

    return nc


_CACHED = None


def _get_compiled():
    global _CACHED
    if _CACHED is None:
        nc = bacc.Bacc("TRN2", target_bir_lowering=False, debug=False,
                       num_devices=N_CORES)
        build(nc)
        nc.compile()
        _CACHED = nc
    return _CACHED


def kernel(**inputs) -> np.ndarray:
    nc = _get_compiled()
    np_in = {k: np.ascontiguousarray(np.asarray(v, dtype=np.float32))
             for k, v in inputs.items()}
    in_maps = []
    for i in range(N_CORES):
        m = dict(np_in)
        m["x_seq"] = np.ascontiguousarray(
            np_in["x_seq"][:, i * BL : (i + 1) * BL])
        in_maps.append(m)
    res = bass_utils.run_bass_kernel_spmd(nc, in_maps, core_ids=list(range(N_CORES)))
    return np.concatenate([res.results[i]["out"] for i in range(N_CORES)], axis=1)


if __name__ == "__main__":
    nc = _get_compiled()
    print("compiled OK")
